# revision 34
# baseline (speedup 1.0000x reference)
"""Trainium2 Bass kernel for windowed sparse attention (nn_Attention_17703855194428).

Reference computation (per window w of 128 = B*X*Y, tokens N=294 = L*W1*W2):
    qkv = x_w @ w_qkv.T ; q,k,v heads (8 heads x 32 dim), q scaled by 1/sqrt(32)
    sim = q @ k.T + rel_pos_bias ; masked cols -> -1e9 ; softmax over keys
    out = (attn @ v) @ w_out.T

Sharding: pure data parallel over the 128 independent windows -> 16 windows
per NeuronCore, weights/bias replicated. No collectives.

Kernel layout strategy (per core, per window; all matmul inputs fp16,
fp32 PSUM accumulate; host pre-computes exp(rel-pos-bias) transposed and the
additive mask in the kernel's SBUF layouts):
    xT [C=256, N=294] (channels on partitions) ->
    q,k as [E, N] (head-dim on partitions), v as [N, E] (tokens on partitions)
    simT[j, i] = sum_d k[d,j] q[d,i] via row-tiled (K=32) matmuls, 2 heads
        per 2-bank psum tile, double-buffered (sim_pairs)
    P_T = exp(simT + mask_j) * exp(bias)_T   (one ACT exp with per-partition
        mask bias reading across psum banks + one DVE fp16 multiply)
    outU.T[hd, i] = sum_j v[j, hd] P_T[j, i] via col-tiled (M=32) matmuls
    rowsum broadcast to all 32 head rows for free via col-tiled ones-matmul
    1/rowsum via DVE reciprocal_approx_fast; normalize is one DVE multiply
    out = (outU.T * (1/rowsum)).T @ w_out.T via K=hd matmuls -> [tok, C],
    written fp16 (host upcasts to fp32)

Measured on HW (differential For_i timing): ~320-344 us per core for 16
windows (run-to-run variance ~7%); 1-stage software-pipelined emission (QKV
of window w+1 ahead of attn of w). PE is the bottleneck: this toolchain
serializes every Ldweights+Matmult pair (--enable-ldw-opt=false, per-MM sem
updates), ~15.5 us/window of serial PE.

Optimization attempts measured SLOWER than this config (kept as flags):
  rs_presum=True   364 us: DVE pt-chunk adds sit on the softmax critical path
  pipe=2           345 us: deeper pipeline thrashes the 2-slot ps1 psum ring
  f32r=True        372 us: f32r (self-loading-weights) qkv/sim/outproj MMs
                   are slower than fp16+Ldweights pairs despite the cost
                   model's >=256-free fast path; numerics better (5.3e-4)
  augv=True        (broken + net-negative): fused rowsum-in-PV via augmented
                   v columns; engine ops cannot read/write different base
                   partitions (verified NaN on HW), so the 1/rowsum
                   broadcast back to head rows needs extra PE/DVE work that
                   cancels the 24 saved ones-matmuls
HW facts verified by probes this session: tile-pool ring slots do keep
warmup-written bytes across later same-tag allocations; DVE/ACT ops with
in/out base-partition mismatch return garbage; DMA cannot read PSUM;
matmul operand base partitions must be equal and in {0,32,64}; engine
partition accesses must start at 0/32/64/96; f32r matmul inputs must be
produced rounded (F32R-dtype outputs of ACT/DVE copies or DMA loads work).
"""

import numpy as np
from contextlib import ExitStack

import concourse.bass as bass
import concourse.bacc as bacc
import concourse.mybir as mybir
from concourse import tile
from concourse.bass_utils import run_bass_kernel_spmd

import ml_dtypes

F32 = mybir.dt.float32
F32R = mybir.dt.float32r
BF16 = mybir.dt.bfloat16
FP16 = mybir.dt.float16
EXP = mybir.ActivationFunctionType.Exp

# Problem constants (hardcoded per harness contract)
B, AGENT, X, Y, WIN, DIM, HEADS, DH = 2, 6, 8, 8, 7, 256, 8, 32
N = AGENT * WIN * WIN            # 294 tokens per window
NWIN = B * X * Y                 # 128 windows
NCORES = 8
WPC = NWIN // NCORES             # 16 windows per core
JC = 98                          # key-chunk size (294 = 3*98)
NJC = 3
SCALE = DH ** -0.5
MASK_NEG = -1e9


def _rel_pos_index(L, Wh, Ww):
    coords = np.stack(np.meshgrid(np.arange(L), np.arange(Wh), np.arange(Ww), indexing="ij"))
    cf = coords.reshape(3, -1)
    rel = cf[:, :, None] - cf[:, None, :]
    rel = rel.transpose(1, 2, 0).astype(np.int64)
    rel[..., 0] += L - 1
    rel[..., 1] += Wh - 1
    rel[..., 2] += Ww - 1
    rel[..., 0] *= (2 * Wh - 1) * (2 * Ww - 1)
    rel[..., 1] *= 2 * Ww - 1
    return rel.sum(-1)  # (N, N) [i, j]


def build_graph(n_wins=WPC, wbufs=2, pvrs_bufs=2, misc_bufs=2, trace_sim=False, reps=1,
                v_eng='act', fo_eng='dve', gp_jc=(), sim_pairs=False, timing=False, out16=False, ab_noexp=False, ab_nosim=False, ab_nopv=False, pipe=False, sim_bufs=2, rs_presum=False,
                augv=False, rrb_sb=False, recip_eng='dve', f32r=False):
    nc = bacc.Bacc(None)
    XDT = F32R if f32r else FP16
    n_xt = 2 if timing else n_wins
    xt_d = nc.declare_dram_parameter("xt", [n_xt, 2, 128, N], XDT, isOutput=False)
    msk_d = nc.declare_dram_parameter("msk", [JC, n_wins * NJC], F32, isOutput=False)
    eb_d = nc.declare_dram_parameter("eb", [JC, HEADS, NJC, N], FP16, isOutput=False)
    wqkv_d = nc.declare_dram_parameter("wqkv", [2, 128, 3 * DIM], XDT, isOutput=False)
    if augv:
        wout_d = nc.declare_dram_parameter("wout4", [4, 128, DIM], FP16, isOutput=False)
    else:
        wout_d = nc.declare_dram_parameter("wout", [2, 128, DIM], XDT, isOutput=False)
    ODT = FP16 if out16 else F32
    if timing:
        out_d = nc.dram_tensor("oscr", [n_wins, N, DIM], ODT)
        outx_d = nc.declare_dram_parameter("out", [1, N, DIM], ODT, isOutput=True)
    else:
        out_d = nc.declare_dram_parameter("out", [n_wins, N, DIM], ODT, isOutput=True)
        outx_d = None

    with tile.TileContext(nc, trace_sim=trace_sim) as tc, ExitStack() as ctx:
        cpool = ctx.enter_context(tc.tile_pool(name="consts", bufs=1))
        wpool = ctx.enter_context(tc.tile_pool(name="work", bufs=wbufs))
        # one xt slot per window: slot reuse on DMA-written tiles piles up
        # sync waits beyond what DMA descriptors support
        xpool = ctx.enter_context(tc.tile_pool(name="xin", bufs=n_wins))
        if sim_pairs:
            psim = ctx.enter_context(tc.tile_pool(name="psim", bufs=sim_bufs, space="PSUM"))
        else:
            psim = ctx.enter_context(tc.tile_pool(name="psim", bufs=1, space="PSUM"))
        pvrs = ctx.enter_context(tc.tile_pool(name="pvrs", bufs=pvrs_bufs, space="PSUM"))
        ps1 = ctx.enter_context(tc.tile_pool(name="ps1", bufs=misc_bufs, space="PSUM"))

        # ---- replicated constants ----
        wqkv_sb = []
        for c in range(2):
            t = cpool.tile([128, 3 * DIM], XDT, tag=f"wqkv{c}")
            nc.sync.dma_start(t[:], wqkv_d[c])
            wqkv_sb.append(t)
        wout_sb = []
        for c in range(4 if augv else 2):
            t = cpool.tile([128, DIM], FP16 if augv else XDT, tag=f"wout{c}")
            nc.sync.dma_start(t[:], wout_d[c])
            wout_sb.append(t)

        eb_sb = cpool.tile([JC, HEADS, NJC, N], FP16, tag="eb")
        nc.sync.dma_start(eb_sb[:], eb_d[:])
        msk_sb = cpool.tile([JC, n_wins * NJC], F32, tag="msk")
        nc.sync.dma_start(msk_sb[:], msk_d[:])
        ones_sb = cpool.tile([JC, 32], FP16, tag="ones")
        nc.vector.memset(ones_sb[:], 1.0)

        if ab_noexp or ab_nosim:
            pt_const = cpool.tile([JC, 4, N], FP16, tag="ptc")
            nc.vector.memset(pt_const[:], 0.01)
            stub = cpool.tile([1, 8], F32, tag="stub")

        # warm-up touches: absorb the one-time const-DMA waits into throwaway
        # instructions so steady-state ops stay within the per-instruction
        # sync-wait budget
        scr_a = cpool.tile([JC, 1], F32, tag="scr_a")
        nc.scalar.copy(scr_a[:], msk_sb[:, 0:1])
        scr_v = cpool.tile([JC, 1], FP16, tag="scr_v")
        nc.vector.tensor_copy(scr_v[:], eb_sb[:, 0, 0, 0:1])

        if augv:
            # one-time init of the v-tile ring slots: cols 32-63 of each head
            # block stay 1.0 forever (per-window copies write only cols 0-31),
            # making the PV matmul's output rows 32-63 the rowsum replicated
            # 32x (free broadcast for the normalize)
            for j in range(NJC):
                for _ in range(wbufs):
                    t = wpool.tile([JC, HEADS, 2 * DH], FP16, tag=f"v{j}")
                    nc.vector.memset(t[:, :, DH:2 * DH], 1.0)
            # one-time zero of the on-tile ring slots' unused rows (32-63,
            # 96-127): they multiply wout4's zero rows in the out-projection
            # and must be finite; per-window muls only write the head rows
            for tg in range(4):
                for _ in range(wbufs):
                    t = wpool.tile([128, N], FP16, tag=f"on{tg}")
                    nc.vector.memset(t[32:64, :], 0.0)
                    nc.vector.memset(t[96:128, :], 0.0)

        def emit_qkv(w):
            xt_t = []
            for c in range(2):
                t = xpool.tile([128, N], XDT, tag=f"xt{c}")
                nc.sync.dma_start(t[:], xt_d[w % n_xt, c])
                xt_t.append(t)
            qk_sb = []
            for p in range(4):
                ps = ps1.tile([128, 512], F32, tag="b1")
                for c in range(2):
                    nc.tensor.matmul(
                        ps[:, 0:N],
                        lhsT=wqkv_sb[c][:, 128 * p:128 * (p + 1)],
                        rhs=xt_t[c][:],
                        start=(c == 0), stop=(c == 1),
                    )
                t = wpool.tile([128, N], XDT, tag=f"qk{p}")
                if p < 2:
                    nc.scalar.copy(t[:], ps[:, 0:N])
                else:
                    nc.vector.tensor_copy(t[:], ps[:, 0:N])
                qk_sb.append(t)
            v_sb = []
            for j in range(NJC):
                ps = ps1.tile([128, 512], F32, tag="b1")
                for c in range(2):
                    nc.tensor.matmul(
                        ps[0:JC, 0:DIM],
                        lhsT=xt_t[c][:, JC * j:JC * (j + 1)],
                        rhs=wqkv_sb[c][:, 2 * DIM:3 * DIM],
                        start=(c == 0), stop=(c == 1),
                    )
                if augv:
                    # [j, head, 64]: cols 0-31 = v channels, cols 32-63 = 1.0
                    # (pre-set once per ring slot) so PV output rows 32-63 are
                    # the softmax rowsum replicated across 32 partitions
                    t = wpool.tile([JC, HEADS, 2 * DH], FP16, tag=f"v{j}")
                    src = ps[0:JC, 0:DIM].rearrange("p (h e) -> p h e", h=HEADS)
                    if v_eng == 'act':
                        nc.scalar.copy(t[:, :, 0:DH], src)
                    else:
                        nc.vector.tensor_copy(t[:, :, 0:DH], src)
                else:
                    t = wpool.tile([JC, DIM], FP16, tag=f"v{j}")
                    if v_eng == 'act':
                        nc.scalar.copy(t[:], ps[0:JC, 0:DIM])
                    else:
                        nc.vector.tensor_copy(t[:], ps[0:JC, 0:DIM])
                v_sb.append(t)
            return qk_sb, v_sb

        def emit_attn(w, qk_sb, v_sb):
            on_sb = []
            for hg in range(2):
                pv = pvrs.tile([128, 512], F32, tag="pvrs")
                rs = pvrs.tile([128, 512], F32, tag="pvrs")
                pts = []
                if ab_nosim:
                    pts = [pt_const] * NJC
                elif sim_pairs:
                    for jc in range(NJC):
                        ptj = wpool.tile([JC, 4, N], FP16, tag=f"pt{jc}")
                        for sg in range(2):
                            smp = psim.tile([128, 1024], F32, tag="sim")
                            for i2 in range(2):
                                t4 = 2 * sg + i2
                                nc.tensor.matmul(
                                    smp[0:JC, 512 * i2:512 * i2 + N],
                                    lhsT=qk_sb[2 + hg][32 * t4:32 * (t4 + 1), JC * jc:JC * (jc + 1)],
                                    rhs=qk_sb[hg][32 * t4:32 * (t4 + 1), :],
                                    start=True, stop=True,
                                    tile_position=(32 * t4, 0),
                                )
                            et = wpool.tile([JC, 2, N], FP16, tag=f"et{jc}{sg}")
                            sim_ap = smp[0:JC, :].rearrange("p (t x) -> p t x", t=2)[:, :, 0:N]
                            nc.scalar.activation(
                                et[:], sim_ap, EXP,
                                bias=msk_sb[:, NJC * w + jc:NJC * w + jc + 1],
                            )
                            eb_ap = eb_sb[:, 4 * hg + 2 * sg:4 * hg + 2 * sg + 2, jc, :]
                            nc.vector.tensor_mul(ptj[:, 2 * sg:2 * sg + 2, :], et[:], eb_ap)
                        pts.append(ptj)
                else:
                    for jc in range(NJC):
                        smp = psim.tile([128, 2048], F32, tag="sim")
                        for t4 in range(4):
                            nc.tensor.matmul(
                                smp[0:JC, 512 * t4:512 * t4 + N],
                                lhsT=qk_sb[2 + hg][32 * t4:32 * (t4 + 1), JC * jc:JC * (jc + 1)],
                                rhs=qk_sb[hg][32 * t4:32 * (t4 + 1), :],
                                start=True, stop=True,
                                tile_position=(32 * t4, 0),
                            )
                        if ab_noexp:
                            nc.scalar.copy(stub[:], smp[0:1, 0:8])
                            pts.append(pt_const)
                            continue
                        et = wpool.tile([JC, 4, N], FP16, tag=f"et{jc}")
                        sim_ap = smp[0:JC, :].rearrange("p (t x) -> p t x", t=4)[:, :, 0:N]
                        nc.scalar.activation(
                            et[:], sim_ap, EXP,
                            bias=msk_sb[:, NJC * w + jc:NJC * w + jc + 1],
                        )
                        pt = wpool.tile([JC, 4, N], FP16, tag=f"pt{jc}")
                        eb_ap = eb_sb[:, 4 * hg:4 * (hg + 1), jc, :]
                        nc.vector.tensor_mul(pt[:], et[:], eb_ap)
                        pts.append(pt)

                ptsum = None
                if rs_presum and not (ab_nopv or ab_nosim or ab_noexp):
                    # rowsum needs sum over all 294 j; sum the three j-chunks
                    # on DVE first so each head needs one ones-matmul, not 3
                    ptsum = wpool.tile([JC, 4, N], FP16, tag="ptsum")
                    nc.vector.tensor_add(ptsum[:], pts[0][:], pts[1][:])
                    nc.vector.tensor_add(ptsum[:], ptsum[:], pts[2][:])
                pv_iters = [(0, [0])] if ab_nopv else [(t, list(range(NJC))) for t in range(4)]
                for t4, jcs in pv_iters:
                    h = 4 * hg + t4
                    for jc in jcs:
                        nc.tensor.matmul(
                            pv[32 * t4:32 * (t4 + 1), 0:N],
                            lhsT=v_sb[jc][:, 32 * h:32 * (h + 1)],
                            rhs=pts[jc][:, t4, :],
                            start=(jc == 0), stop=(jc == jcs[-1]),
                            tile_position=(0, 32 * t4),
                            skip_group_check=True,
                        )
                    if ptsum is not None:
                        nc.tensor.matmul(
                            rs[32 * t4:32 * (t4 + 1), 0:N],
                            lhsT=ones_sb[:],
                            rhs=ptsum[:, t4, :],
                            start=True, stop=True,
                            tile_position=(0, 32 * t4),
                            skip_group_check=True,
                        )
                        continue
                    for jc in jcs:
                        nc.tensor.matmul(
                            rs[32 * t4:32 * (t4 + 1), 0:N],
                            lhsT=ones_sb[:],
                            rhs=pts[jc][:, t4, :],
                            start=(jc == 0), stop=(jc == jcs[-1]),
                            tile_position=(0, 32 * t4),
                            skip_group_check=True,
                        )
                rr = wpool.tile([128, N], F32, tag="rr")
                nc.vector.reciprocal_approx_fast(rr[:], rs[:, 0:N])
                on = wpool.tile([128, N], XDT, tag=f"on{hg}")
                nc.vector.tensor_mul(on[:], pv[:, 0:N], rr[:])
                on_sb.append(on)

            for ic in range(NJC):
                po = ps1.tile([128, 512], F32, tag="b1")
                for kc in range(2):
                    nc.tensor.matmul(
                        po[0:JC, 0:DIM],
                        lhsT=on_sb[kc][:, JC * ic:JC * (ic + 1)],
                        rhs=wout_sb[kc][:],
                        start=(kc == 0), stop=(kc == 1),
                    )
                fo = wpool.tile([JC, DIM], FP16 if out16 else F32, tag=f"fo{ic}")
                if fo_eng == 'dve':
                    nc.vector.tensor_copy(fo[:], po[0:JC, 0:DIM])
                else:
                    nc.scalar.copy(fo[:], po[0:JC, 0:DIM])
                nc.sync.dma_start(out_d[w, JC * ic:JC * (ic + 1), :], fo[:])
                if timing and w == 0:
                    nc.sync.dma_start(outx_d[0, JC * ic:JC * (ic + 1), :], fo[:])

        def emit_attn_augv(w, qk_sb, v_sb):
            """PV matmuls also produce the softmax rowsum (augmented-v 33rd
            column); no ones-matmul rowsum pass. Head pair per psum tile at
            col positions 0/64; rowsums land on partitions 32/96."""
            on_sb = [None] * 4
            for hg in range(2):
                pts = []
                for jc in range(NJC):
                    ptj = wpool.tile([JC, 4, N], FP16, tag=f"pt{jc}")
                    for sg in range(2):
                        smp = psim.tile([128, 1024], F32, tag="sim")
                        for i2 in range(2):
                            t4 = 2 * sg + i2
                            nc.tensor.matmul(
                                smp[0:JC, 512 * i2:512 * i2 + N],
                                lhsT=qk_sb[2 + hg][32 * t4:32 * (t4 + 1), JC * jc:JC * (jc + 1)],
                                rhs=qk_sb[hg][32 * t4:32 * (t4 + 1), :],
                                start=True, stop=True,
                                tile_position=(32 * t4, 0),
                            )
                        et = wpool.tile([JC, 2, N], FP16, tag=f"et{jc}{sg}")
                        sim_ap = smp[0:JC, :].rearrange("p (t x) -> p t x", t=2)[:, :, 0:N]
                        nc.scalar.activation(
                            et[:], sim_ap, EXP,
                            bias=msk_sb[:, NJC * w + jc:NJC * w + jc + 1],
                        )
                        eb_ap = eb_sb[:, 4 * hg + 2 * sg:4 * hg + 2 * sg + 2, jc, :]
                        nc.vector.tensor_mul(ptj[:, 2 * sg:2 * sg + 2, :], et[:], eb_ap)
                    pts.append(ptj)

                for tl in range(2):
                    pv = pvrs.tile([128, 512], F32, tag="pvrs")
                    for hp in range(2):
                        hl = 2 * tl + hp
                        hglob = 4 * hg + hl
                        for jc in range(NJC):
                            nc.tensor.matmul(
                                pv[64 * hp:64 * (hp + 1), 0:N],
                                lhsT=v_sb[jc][:, hglob, :],
                                rhs=pts[jc][:, hl, :],
                                start=(jc == 0), stop=(jc == NJC - 1),
                                tile_position=(0, 64 * hp),
                                skip_group_check=True,
                            )
                    # pv rows: 0-31 headA out, 32-63 headA rowsum (x32),
                    #          64-95 headB out, 96-127 headB rowsum (x32)
                    tg = 2 * hg + tl
                    rb = wpool.tile([96, N], F32, tag=f"rb{tg}")
                    nc.vector.reciprocal_approx_fast(rb[0:32, :], pv[32:64, 0:N])
                    nc.vector.reciprocal_approx_fast(rb[64:96, :], pv[96:128, 0:N])
                    on = wpool.tile([128, N], FP16, tag=f"on{tg}")
                    nc.vector.tensor_mul(on[0:32, :], pv[0:32, 0:N], rb[0:32, :])
                    nc.vector.tensor_mul(on[64:96, :], pv[64:96, 0:N], rb[64:96, :])
                    on_sb[tg] = on

            for ic in range(NJC):
                po = ps1.tile([128, 512], F32, tag="b1")
                for tg in range(4):
                    nc.tensor.matmul(
                        po[0:JC, 0:DIM],
                        lhsT=on_sb[tg][:, JC * ic:JC * (ic + 1)],
                        rhs=wout_sb[tg][:],
                        start=(tg == 0), stop=(tg == 3),
                    )
                fo = wpool.tile([JC, DIM], FP16 if out16 else F32, tag=f"fo{ic}")
                if fo_eng == 'dve':
                    nc.vector.tensor_copy(fo[:], po[0:JC, 0:DIM])
                else:
                    nc.scalar.copy(fo[:], po[0:JC, 0:DIM])
                nc.sync.dma_start(out_d[w, JC * ic:JC * (ic + 1), :], fo[:])
                if timing and w == 0:
                    nc.sync.dma_start(outx_d[0, JC * ic:JC * (ic + 1), :], fo[:])

        if augv:
            emit_attn = emit_attn_augv

        rep_ctx = tc.For_i(0, reps, 1) if reps > 1 else None
        if rep_ctx is not None:
            ctx.enter_context(rep_ctx)
        if pipe:
            # software pipeline: QKV of windows w+1..w+pipe emitted (and thus
            # prioritized) ahead of attention of window w, so the PE fills
            # softmax-chain stalls with upcoming windows' projections
            depth = int(pipe)
            inflight = []
            for w in range(n_wins):
                inflight.append(emit_qkv(w))
                if len(inflight) > depth:
                    emit_attn(w - depth, *inflight.pop(0))
            for i, qv in enumerate(inflight):
                emit_attn(n_wins - len(inflight) + i, *qv)
        else:
            for w in range(n_wins):
                qk_sb, v_sb = emit_qkv(w)
                emit_attn(w, qk_sb, v_sb)

    nc.compile()
    return nc


def host_prep(x, mask, w_qkv, w_out, bias_table, f32r=None):
    """Build per-core input maps (numpy only)."""
    if f32r is None:
        f32r = bool(BEST_CFG.get("f32r", False))
    xdt = np.float32 if f32r else np.float16
    x = np.asarray(x, dtype=np.float32)
    mask = np.asarray(mask)
    w_qkv = np.asarray(w_qkv, dtype=np.float32)
    w_out = np.asarray(w_out, dtype=np.float32)
    bias_table = np.asarray(bias_table, dtype=np.float32)

    # x: (B, L, X, Y, W1, W2, C) -> windows (B,X,Y) x [C, N]
    xr = np.ascontiguousarray(x.transpose(0, 2, 3, 1, 4, 5, 6)).reshape(NWIN, N, DIM)
    xt = np.ascontiguousarray(xr.transpose(0, 2, 1)).reshape(NWIN, 2, 128, N).astype(xdt)

    # mask: (B, X, Y, W1, W2, 1, L) -> (B,X,Y) x N with token order (l, w1, w2)
    m = np.ascontiguousarray(mask.transpose(0, 1, 2, 5, 6, 3, 4)).reshape(NWIN, N)
    maskadd = np.where(m == 0, np.float32(MASK_NEG), np.float32(0.0)).astype(np.float32)

    # exp(bias) transposed: ebT[h, j, i] = exp(bias[i, j, h])
    ri = _rel_pos_index(AGENT, WIN, WIN)
    bias = bias_table[ri]                       # (N, N, H) [i, j, h]
    ebT = np.exp(bias.transpose(2, 1, 0))       # (H, j, i)
    eb_host = np.ascontiguousarray(
        ebT.reshape(HEADS, NJC, JC, N).transpose(2, 0, 1, 3)
    ).astype(np.float16)                # (JC, H, NJC, N)

    wq = w_qkv.copy()
    wq[0:DIM] *= np.float32(SCALE)
    wqkvT = np.ascontiguousarray(wq.T).reshape(2, 128, 3 * DIM).astype(xdt)
    woutT = np.ascontiguousarray(w_out.T).reshape(2, 128, DIM).astype(xdt)

    # sparse head layout for augv: tile T rows 0-31 = head 2T, rows 64-95 =
    # head 2T+1, other rows zero (multiplied by on-rows that are zero)
    wt = np.ascontiguousarray(w_out.T)          # [e, c']
    wout4 = np.zeros((4, 128, DIM), np.float32)
    for t in range(4):
        wout4[t, 0:32] = wt[64 * t:64 * t + 32]
        wout4[t, 64:96] = wt[64 * t + 32:64 * t + 64]
    wout4 = wout4.astype(np.float16)

    in_maps = []
    for core in range(NCORES):
        ws = slice(WPC * core, WPC * (core + 1))
        mm = maskadd[ws].reshape(WPC, NJC, JC).transpose(2, 0, 1).reshape(JC, WPC * NJC)
        in_maps.append({
            "xt": np.ascontiguousarray(xt[ws]),
            "msk": np.ascontiguousarray(mm),
            "eb": eb_host,
            "wqkv": wqkvT,
            "wout": woutT,
            "wout4": wout4,
        })
    return in_maps


def assemble_output(core_outs):
    """core_outs: list of [WPC, N, DIM] arrays -> full (B, L, X, Y, W1, W2, C)."""
    out = np.concatenate([np.asarray(o) for o in core_outs], axis=0).astype(np.float32)
    out = out.reshape(B, X, Y, AGENT, WIN, WIN, DIM)
    return np.ascontiguousarray(out.transpose(0, 3, 1, 2, 4, 5, 6)).astype(np.float32)


_NC_CACHE = {}


def _get_nc(n_wins=WPC):
    if n_wins not in _NC_CACHE:
        _NC_CACHE[n_wins] = build_graph(n_wins, **BEST_CFG)
    return _NC_CACHE[n_wins]


BEST_CFG = dict(wbufs=4, v_eng="dve", out16=True, sim_pairs=True, fo_eng="act", pipe=True)


def kernel(x, mask, w_qkv, w_out, bias_table):
    in_maps = host_prep(x, mask, w_qkv, w_out, bias_table)
    nc = _get_nc(WPC)
    res = run_bass_kernel_spmd(nc, in_maps, core_ids=list(range(NCORES)))
    core_outs = [res.results[i]["out"] for i in range(NCORES)]
    return assemble_output(core_outs)



# revision 40
# speedup vs baseline: 1.2341x; 1.2341x over previous
"""Trainium2 Bass kernel for windowed sparse attention (nn_Attention_17703855194428).

Reference computation (per window w of 128 = B*X*Y, tokens N=294 = L*W1*W2):
    qkv = x_w @ w_qkv.T ; q,k,v heads (8 heads x 32 dim), q scaled by 1/sqrt(32)
    sim = q @ k.T + rel_pos_bias ; masked cols -> -1e9 ; softmax over keys
    out = (attn @ v) @ w_out.T

Sharding: pure data parallel over the 128 independent windows -> 16 windows
per NeuronCore, weights/bias replicated. No collectives.

kcomp (key compaction, BEST_CFG default): ~50% of keys are masked per
window (mask ~ Bernoulli(0.5) over the 294 key slots), and masked keys
contribute exactly exp(-1e9)=0. The host reorders each window's keys
(unmasked first, masked as pad) so the live keys fit in ceil(nk/98)=2
chunks instead of 3: sim/PV/rowsum matmuls drop 24->16 each, exp 12->8,
eb-muls 12->8, v-proj 6->4 per window (92 -> 66 matmuls total, -30% PE
stream, -33% ACT/DVE). Queries keep all 294 tokens and original order, so
the output layout is unchanged. Per-window compacted exp-bias tables
(ebw, 922KB/window) are streamed per window through a 3-deep SBUF ring
instead of one shared table; the additive mask bias now encodes compacted
position >= nk (pad) rather than the original mask. All 8 cores run one
SPMD instruction stream, so each window slot's chunk count is the max
over cores (still 2 for this input distribution, nk ~ 147 +- 9).

Kernel layout strategy (per core, per window; all matmul inputs fp16,
fp32 PSUM accumulate; host pre-computes exp(rel-pos-bias) transposed and the
additive mask in the kernel's SBUF layouts):
    xT [C=256, N=294] (channels on partitions) ->
    q,k as [E, N] (head-dim on partitions), v as [N, E] (tokens on partitions)
    simT[j, i] = sum_d k[d,j] q[d,i] via row-tiled (K=32) matmuls, 2 heads
        per 2-bank psum tile, double-buffered (sim_pairs)
    P_T = exp(simT + mask_j) * exp(bias)_T   (one ACT exp with per-partition
        mask bias reading across psum banks + one DVE fp16 multiply)
    outU.T[hd, i] = sum_j v[j, hd] P_T[j, i] via col-tiled (M=32) matmuls
    rowsum broadcast to all 32 head rows for free via col-tiled ones-matmul
    1/rowsum via DVE reciprocal_approx_fast; normalize is one DVE multiply
    out = (outU.T * (1/rowsum)).T @ w_out.T via K=hd matmuls -> [tok, C],
    written fp16 (host upcasts to fp32)

Measured on HW (differential For_i timing): 319-410 us per core for 16
windows for THIS config across one session — run-to-run variance on the
shared axon TRN2 is large (up to ~25%), so single-run deltas under ~10% are
not trustworthy; 1-stage software-pipelined emission (QKV of window w+1
ahead of attn of w). PE is the bottleneck: this toolchain
serializes every Ldweights+Matmult pair (--enable-ldw-opt=false, per-MM sem
updates), ~15.5 us/window of serial PE.

Optimization attempts measured SLOWER than this config (kept as flags):
  rs_presum=True   364 us: DVE pt-chunk adds sit on the softmax critical path
  pipe=2           345 us: deeper pipeline thrashes the 2-slot ps1 psum ring
  f32r=True        372 us: f32r (self-loading-weights) qkv/sim/outproj MMs
                   are slower than fp16+Ldweights pairs despite the cost
                   model's >=256-free fast path; numerics better (5.3e-4)
  augv=True        (broken + net-negative): fused rowsum-in-PV via augmented
                   v columns; engine ops cannot read/write different base
                   partitions (verified NaN on HW), so the 1/rowsum
                   broadcast back to head rows needs extra PE/DVE work that
                   cancels the 24 saved ones-matmuls
HW facts verified by probes this session: tile-pool ring slots do keep
warmup-written bytes across later same-tag allocations; DVE/ACT ops with
in/out base-partition mismatch return garbage; DMA cannot read PSUM;
matmul operand base partitions must be equal and in {0,32,64}; engine
partition accesses must start at 0/32/64/96; f32r matmul inputs must be
produced rounded (F32R-dtype outputs of ACT/DVE copies or DMA loads work).
"""

import numpy as np
from contextlib import ExitStack

import concourse.bass as bass
import concourse.bacc as bacc
import concourse.mybir as mybir
from concourse import tile
from concourse.bass_utils import run_bass_kernel_spmd

import ml_dtypes

F32 = mybir.dt.float32
F32R = mybir.dt.float32r
BF16 = mybir.dt.bfloat16
FP16 = mybir.dt.float16
EXP = mybir.ActivationFunctionType.Exp

# Problem constants (hardcoded per harness contract)
B, AGENT, X, Y, WIN, DIM, HEADS, DH = 2, 6, 8, 8, 7, 256, 8, 32
N = AGENT * WIN * WIN            # 294 tokens per window
NWIN = B * X * Y                 # 128 windows
NCORES = 8
WPC = NWIN // NCORES             # 16 windows per core
JC = 98                          # key-chunk size (294 = 3*98)
NJC = 3
SCALE = DH ** -0.5
MASK_NEG = -1e9


def _rel_pos_index(L, Wh, Ww):
    coords = np.stack(np.meshgrid(np.arange(L), np.arange(Wh), np.arange(Ww), indexing="ij"))
    cf = coords.reshape(3, -1)
    rel = cf[:, :, None] - cf[:, None, :]
    rel = rel.transpose(1, 2, 0).astype(np.int64)
    rel[..., 0] += L - 1
    rel[..., 1] += Wh - 1
    rel[..., 2] += Ww - 1
    rel[..., 0] *= (2 * Wh - 1) * (2 * Ww - 1)
    rel[..., 1] *= 2 * Ww - 1
    return rel.sum(-1)  # (N, N) [i, j]


def build_graph(n_wins=WPC, wbufs=2, pvrs_bufs=2, misc_bufs=2, trace_sim=False, reps=1,
                v_eng='act', fo_eng='dve', gp_jc=(), sim_pairs=False, timing=False, out16=False, ab_noexp=False, ab_nosim=False, ab_nopv=False, pipe=False, sim_bufs=2, rs_presum=False,
                augv=False, rrb_sb=False, recip_eng='dve', f32r=False,
                kcomp=False, nkc=None):
    nc = bacc.Bacc(None)
    XDT = F32R if f32r else FP16
    nkc_l = list(nkc) if nkc else [NJC] * n_wins
    NKMAX = max(nkc_l)
    KPAD = JC * NKMAX
    n_xt = 2 if timing else n_wins
    xt_d = nc.declare_dram_parameter("xt", [n_xt, 2, 128, N], XDT, isOutput=False)
    if kcomp:
        # mask-compacted keys: per window only ceil(nk/98) chunks of keys
        # (unmasked tokens first, masked-as-pad after) take part in
        # sim/softmax/PV/rowsum; queries stay all N tokens
        xtk_d = nc.declare_dram_parameter("xtk", [n_xt, 2, 128, KPAD], XDT, isOutput=False)
        ebw_d = nc.declare_dram_parameter("ebw", [n_xt, JC, HEADS, NKMAX, N], FP16, isOutput=False)
    msk_d = nc.declare_dram_parameter("msk", [JC, n_wins * NJC], F32, isOutput=False)
    eb_d = None
    if not kcomp:
        eb_d = nc.declare_dram_parameter("eb", [JC, HEADS, NJC, N], FP16, isOutput=False)
    wqkv_d = nc.declare_dram_parameter("wqkv", [2, 128, 3 * DIM], XDT, isOutput=False)
    if augv:
        wout_d = nc.declare_dram_parameter("wout4", [4, 128, DIM], FP16, isOutput=False)
    else:
        wout_d = nc.declare_dram_parameter("wout", [2, 128, DIM], XDT, isOutput=False)
    ODT = FP16 if out16 else F32
    if timing:
        out_d = nc.dram_tensor("oscr", [n_wins, N, DIM], ODT)
        outx_d = nc.declare_dram_parameter("out", [1, N, DIM], ODT, isOutput=True)
    else:
        out_d = nc.declare_dram_parameter("out", [n_wins, N, DIM], ODT, isOutput=True)
        outx_d = None

    with tile.TileContext(nc, trace_sim=trace_sim) as tc, ExitStack() as ctx:
        cpool = ctx.enter_context(tc.tile_pool(name="consts", bufs=1))
        wpool = ctx.enter_context(tc.tile_pool(name="work", bufs=wbufs))
        # one xt slot per window: slot reuse on DMA-written tiles piles up
        # sync waits beyond what DMA descriptors support
        xpool = ctx.enter_context(tc.tile_pool(name="xin", bufs=n_wins))
        if sim_pairs:
            psim = ctx.enter_context(tc.tile_pool(name="psim", bufs=sim_bufs, space="PSUM"))
        else:
            psim = ctx.enter_context(tc.tile_pool(name="psim", bufs=1, space="PSUM"))
        pvrs = ctx.enter_context(tc.tile_pool(name="pvrs", bufs=pvrs_bufs, space="PSUM"))
        ps1 = ctx.enter_context(tc.tile_pool(name="ps1", bufs=misc_bufs, space="PSUM"))
        if kcomp:
            ebpool = ctx.enter_context(tc.tile_pool(name="ebp", bufs=3))

        # ---- replicated constants ----
        wqkv_sb = []
        for c in range(2):
            t = cpool.tile([128, 3 * DIM], XDT, tag=f"wqkv{c}")
            nc.sync.dma_start(t[:], wqkv_d[c])
            wqkv_sb.append(t)
        wout_sb = []
        for c in range(4 if augv else 2):
            t = cpool.tile([128, DIM], FP16 if augv else XDT, tag=f"wout{c}")
            nc.sync.dma_start(t[:], wout_d[c])
            wout_sb.append(t)

        if not kcomp:
            eb_sb = cpool.tile([JC, HEADS, NJC, N], FP16, tag="eb")
            nc.sync.dma_start(eb_sb[:], eb_d[:])
        msk_sb = cpool.tile([JC, n_wins * NJC], F32, tag="msk")
        nc.sync.dma_start(msk_sb[:], msk_d[:])
        ones_sb = cpool.tile([JC, 32], FP16, tag="ones")
        nc.vector.memset(ones_sb[:], 1.0)

        if ab_noexp or ab_nosim:
            pt_const = cpool.tile([JC, 4, N], FP16, tag="ptc")
            nc.vector.memset(pt_const[:], 0.01)
            stub = cpool.tile([1, 8], F32, tag="stub")

        # warm-up touches: absorb the one-time const-DMA waits into throwaway
        # instructions so steady-state ops stay within the per-instruction
        # sync-wait budget
        scr_a = cpool.tile([JC, 1], F32, tag="scr_a")
        nc.scalar.copy(scr_a[:], msk_sb[:, 0:1])
        scr_v = cpool.tile([JC, 1], FP16, tag="scr_v")
        if kcomp:
            nc.vector.tensor_copy(scr_v[:], wqkv_sb[0][0:JC, 0:1])
        else:
            nc.vector.tensor_copy(scr_v[:], eb_sb[:, 0, 0, 0:1])

        if augv:
            # one-time init of the v-tile ring slots: cols 32-63 of each head
            # block stay 1.0 forever (per-window copies write only cols 0-31),
            # making the PV matmul's output rows 32-63 the rowsum replicated
            # 32x (free broadcast for the normalize)
            for j in range(NJC):
                for _ in range(wbufs):
                    t = wpool.tile([JC, HEADS, 2 * DH], FP16, tag=f"v{j}")
                    nc.vector.memset(t[:, :, DH:2 * DH], 1.0)
            # one-time zero of the on-tile ring slots' unused rows (32-63,
            # 96-127): they multiply wout4's zero rows in the out-projection
            # and must be finite; per-window muls only write the head rows
            for tg in range(4):
                for _ in range(wbufs):
                    t = wpool.tile([128, N], FP16, tag=f"on{tg}")
                    nc.vector.memset(t[32:64, :], 0.0)
                    nc.vector.memset(t[96:128, :], 0.0)

        def emit_qkv(w):
            xt_t = []
            for c in range(2):
                t = xpool.tile([128, N], XDT, tag=f"xt{c}")
                nc.sync.dma_start(t[:], xt_d[w % n_xt, c])
                xt_t.append(t)
            qk_sb = []
            for p in range(4):
                ps = ps1.tile([128, 512], F32, tag="b1")
                for c in range(2):
                    nc.tensor.matmul(
                        ps[:, 0:N],
                        lhsT=wqkv_sb[c][:, 128 * p:128 * (p + 1)],
                        rhs=xt_t[c][:],
                        start=(c == 0), stop=(c == 1),
                    )
                t = wpool.tile([128, N], XDT, tag=f"qk{p}")
                if p < 2:
                    nc.scalar.copy(t[:], ps[:, 0:N])
                else:
                    nc.vector.tensor_copy(t[:], ps[:, 0:N])
                qk_sb.append(t)
            v_sb = []
            for j in range(NJC):
                ps = ps1.tile([128, 512], F32, tag="b1")
                for c in range(2):
                    nc.tensor.matmul(
                        ps[0:JC, 0:DIM],
                        lhsT=xt_t[c][:, JC * j:JC * (j + 1)],
                        rhs=wqkv_sb[c][:, 2 * DIM:3 * DIM],
                        start=(c == 0), stop=(c == 1),
                    )
                if augv:
                    # [j, head, 64]: cols 0-31 = v channels, cols 32-63 = 1.0
                    # (pre-set once per ring slot) so PV output rows 32-63 are
                    # the softmax rowsum replicated across 32 partitions
                    t = wpool.tile([JC, HEADS, 2 * DH], FP16, tag=f"v{j}")
                    src = ps[0:JC, 0:DIM].rearrange("p (h e) -> p h e", h=HEADS)
                    if v_eng == 'act':
                        nc.scalar.copy(t[:, :, 0:DH], src)
                    else:
                        nc.vector.tensor_copy(t[:, :, 0:DH], src)
                else:
                    t = wpool.tile([JC, DIM], FP16, tag=f"v{j}")
                    if v_eng == 'act':
                        nc.scalar.copy(t[:], ps[0:JC, 0:DIM])
                    else:
                        nc.vector.tensor_copy(t[:], ps[0:JC, 0:DIM])
                v_sb.append(t)
            return qk_sb, v_sb

        def emit_attn(w, qk_sb, v_sb):
            on_sb = []
            for hg in range(2):
                pv = pvrs.tile([128, 512], F32, tag="pvrs")
                rs = pvrs.tile([128, 512], F32, tag="pvrs")
                pts = []
                if ab_nosim:
                    pts = [pt_const] * NJC
                elif sim_pairs:
                    for jc in range(NJC):
                        ptj = wpool.tile([JC, 4, N], FP16, tag=f"pt{jc}")
                        for sg in range(2):
                            smp = psim.tile([128, 1024], F32, tag="sim")
                            for i2 in range(2):
                                t4 = 2 * sg + i2
                                nc.tensor.matmul(
                                    smp[0:JC, 512 * i2:512 * i2 + N],
                                    lhsT=qk_sb[2 + hg][32 * t4:32 * (t4 + 1), JC * jc:JC * (jc + 1)],
                                    rhs=qk_sb[hg][32 * t4:32 * (t4 + 1), :],
                                    start=True, stop=True,
                                    tile_position=(32 * t4, 0),
                                )
                            et = wpool.tile([JC, 2, N], FP16, tag=f"et{jc}{sg}")
                            sim_ap = smp[0:JC, :].rearrange("p (t x) -> p t x", t=2)[:, :, 0:N]
                            nc.scalar.activation(
                                et[:], sim_ap, EXP,
                                bias=msk_sb[:, NJC * w + jc:NJC * w + jc + 1],
                            )
                            eb_ap = eb_sb[:, 4 * hg + 2 * sg:4 * hg + 2 * sg + 2, jc, :]
                            nc.vector.tensor_mul(ptj[:, 2 * sg:2 * sg + 2, :], et[:], eb_ap)
                        pts.append(ptj)
                else:
                    for jc in range(NJC):
                        smp = psim.tile([128, 2048], F32, tag="sim")
                        for t4 in range(4):
                            nc.tensor.matmul(
                                smp[0:JC, 512 * t4:512 * t4 + N],
                                lhsT=qk_sb[2 + hg][32 * t4:32 * (t4 + 1), JC * jc:JC * (jc + 1)],
                                rhs=qk_sb[hg][32 * t4:32 * (t4 + 1), :],
                                start=True, stop=True,
                                tile_position=(32 * t4, 0),
                            )
                        if ab_noexp:
                            nc.scalar.copy(stub[:], smp[0:1, 0:8])
                            pts.append(pt_const)
                            continue
                        et = wpool.tile([JC, 4, N], FP16, tag=f"et{jc}")
                        sim_ap = smp[0:JC, :].rearrange("p (t x) -> p t x", t=4)[:, :, 0:N]
                        nc.scalar.activation(
                            et[:], sim_ap, EXP,
                            bias=msk_sb[:, NJC * w + jc:NJC * w + jc + 1],
                        )
                        pt = wpool.tile([JC, 4, N], FP16, tag=f"pt{jc}")
                        eb_ap = eb_sb[:, 4 * hg:4 * (hg + 1), jc, :]
                        nc.vector.tensor_mul(pt[:], et[:], eb_ap)
                        pts.append(pt)

                ptsum = None
                if rs_presum and not (ab_nopv or ab_nosim or ab_noexp):
                    # rowsum needs sum over all 294 j; sum the three j-chunks
                    # on DVE first so each head needs one ones-matmul, not 3
                    ptsum = wpool.tile([JC, 4, N], FP16, tag="ptsum")
                    nc.vector.tensor_add(ptsum[:], pts[0][:], pts[1][:])
                    nc.vector.tensor_add(ptsum[:], ptsum[:], pts[2][:])
                pv_iters = [(0, [0])] if ab_nopv else [(t, list(range(NJC))) for t in range(4)]
                for t4, jcs in pv_iters:
                    h = 4 * hg + t4
                    for jc in jcs:
                        nc.tensor.matmul(
                            pv[32 * t4:32 * (t4 + 1), 0:N],
                            lhsT=v_sb[jc][:, 32 * h:32 * (h + 1)],
                            rhs=pts[jc][:, t4, :],
                            start=(jc == 0), stop=(jc == jcs[-1]),
                            tile_position=(0, 32 * t4),
                            skip_group_check=True,
                        )
                    if ptsum is not None:
                        nc.tensor.matmul(
                            rs[32 * t4:32 * (t4 + 1), 0:N],
                            lhsT=ones_sb[:],
                            rhs=ptsum[:, t4, :],
                            start=True, stop=True,
                            tile_position=(0, 32 * t4),
                            skip_group_check=True,
                        )
                        continue
                    for jc in jcs:
                        nc.tensor.matmul(
                            rs[32 * t4:32 * (t4 + 1), 0:N],
                            lhsT=ones_sb[:],
                            rhs=pts[jc][:, t4, :],
                            start=(jc == 0), stop=(jc == jcs[-1]),
                            tile_position=(0, 32 * t4),
                            skip_group_check=True,
                        )
                rr = wpool.tile([128, N], F32, tag="rr")
                nc.vector.reciprocal_approx_fast(rr[:], rs[:, 0:N])
                on = wpool.tile([128, N], XDT, tag=f"on{hg}")
                nc.vector.tensor_mul(on[:], pv[:, 0:N], rr[:])
                on_sb.append(on)

            for ic in range(NJC):
                po = ps1.tile([128, 512], F32, tag="b1")
                for kc in range(2):
                    nc.tensor.matmul(
                        po[0:JC, 0:DIM],
                        lhsT=on_sb[kc][:, JC * ic:JC * (ic + 1)],
                        rhs=wout_sb[kc][:],
                        start=(kc == 0), stop=(kc == 1),
                    )
                fo = wpool.tile([JC, DIM], FP16 if out16 else F32, tag=f"fo{ic}")
                if fo_eng == 'dve':
                    nc.vector.tensor_copy(fo[:], po[0:JC, 0:DIM])
                else:
                    nc.scalar.copy(fo[:], po[0:JC, 0:DIM])
                nc.sync.dma_start(out_d[w, JC * ic:JC * (ic + 1), :], fo[:])
                if timing and w == 0:
                    nc.sync.dma_start(outx_d[0, JC * ic:JC * (ic + 1), :], fo[:])

        def emit_attn_augv(w, qk_sb, v_sb):
            """PV matmuls also produce the softmax rowsum (augmented-v 33rd
            column); no ones-matmul rowsum pass. Head pair per psum tile at
            col positions 0/64; rowsums land on partitions 32/96."""
            on_sb = [None] * 4
            for hg in range(2):
                pts = []
                for jc in range(NJC):
                    ptj = wpool.tile([JC, 4, N], FP16, tag=f"pt{jc}")
                    for sg in range(2):
                        smp = psim.tile([128, 1024], F32, tag="sim")
                        for i2 in range(2):
                            t4 = 2 * sg + i2
                            nc.tensor.matmul(
                                smp[0:JC, 512 * i2:512 * i2 + N],
                                lhsT=qk_sb[2 + hg][32 * t4:32 * (t4 + 1), JC * jc:JC * (jc + 1)],
                                rhs=qk_sb[hg][32 * t4:32 * (t4 + 1), :],
                                start=True, stop=True,
                                tile_position=(32 * t4, 0),
                            )
                        et = wpool.tile([JC, 2, N], FP16, tag=f"et{jc}{sg}")
                        sim_ap = smp[0:JC, :].rearrange("p (t x) -> p t x", t=2)[:, :, 0:N]
                        nc.scalar.activation(
                            et[:], sim_ap, EXP,
                            bias=msk_sb[:, NJC * w + jc:NJC * w + jc + 1],
                        )
                        eb_ap = eb_sb[:, 4 * hg + 2 * sg:4 * hg + 2 * sg + 2, jc, :]
                        nc.vector.tensor_mul(ptj[:, 2 * sg:2 * sg + 2, :], et[:], eb_ap)
                    pts.append(ptj)

                for tl in range(2):
                    pv = pvrs.tile([128, 512], F32, tag="pvrs")
                    for hp in range(2):
                        hl = 2 * tl + hp
                        hglob = 4 * hg + hl
                        for jc in range(NJC):
                            nc.tensor.matmul(
                                pv[64 * hp:64 * (hp + 1), 0:N],
                                lhsT=v_sb[jc][:, hglob, :],
                                rhs=pts[jc][:, hl, :],
                                start=(jc == 0), stop=(jc == NJC - 1),
                                tile_position=(0, 64 * hp),
                                skip_group_check=True,
                            )
                    # pv rows: 0-31 headA out, 32-63 headA rowsum (x32),
                    #          64-95 headB out, 96-127 headB rowsum (x32)
                    tg = 2 * hg + tl
                    rb = wpool.tile([96, N], F32, tag=f"rb{tg}")
                    nc.vector.reciprocal_approx_fast(rb[0:32, :], pv[32:64, 0:N])
                    nc.vector.reciprocal_approx_fast(rb[64:96, :], pv[96:128, 0:N])
                    on = wpool.tile([128, N], FP16, tag=f"on{tg}")
                    nc.vector.tensor_mul(on[0:32, :], pv[0:32, 0:N], rb[0:32, :])
                    nc.vector.tensor_mul(on[64:96, :], pv[64:96, 0:N], rb[64:96, :])
                    on_sb[tg] = on

            for ic in range(NJC):
                po = ps1.tile([128, 512], F32, tag="b1")
                for tg in range(4):
                    nc.tensor.matmul(
                        po[0:JC, 0:DIM],
                        lhsT=on_sb[tg][:, JC * ic:JC * (ic + 1)],
                        rhs=wout_sb[tg][:],
                        start=(tg == 0), stop=(tg == 3),
                    )
                fo = wpool.tile([JC, DIM], FP16 if out16 else F32, tag=f"fo{ic}")
                if fo_eng == 'dve':
                    nc.vector.tensor_copy(fo[:], po[0:JC, 0:DIM])
                else:
                    nc.scalar.copy(fo[:], po[0:JC, 0:DIM])
                nc.sync.dma_start(out_d[w, JC * ic:JC * (ic + 1), :], fo[:])
                if timing and w == 0:
                    nc.sync.dma_start(outx_d[0, JC * ic:JC * (ic + 1), :], fo[:])

        def emit_qkv_kc(w):
            nk_c = nkc_l[w]
            kp = JC * nk_c
            xt_t = []
            for c in range(2):
                t = xpool.tile([128, N], XDT, tag=f"xt{c}")
                nc.sync.dma_start(t[:], xt_d[w % n_xt, c])
                xt_t.append(t)
            xtk_t = []
            for c in range(2):
                t = xpool.tile([128, KPAD], XDT, tag=f"xtk{c}")
                nc.sync.dma_start(t[:, 0:kp], xtk_d[w % n_xt, c, :, 0:kp])
                xtk_t.append(t)
            ebw_t = ebpool.tile([JC, HEADS, NKMAX, N], FP16, tag="ebw")
            nc.sync.dma_start(ebw_t[:, :, 0:nk_c, :], ebw_d[w % n_xt, :, :, 0:nk_c, :])
            qk_sb = []
            for p in range(4):
                ps = ps1.tile([128, 512], F32, tag="b1")
                if p < 2:
                    for c in range(2):
                        nc.tensor.matmul(
                            ps[:, 0:N],
                            lhsT=wqkv_sb[c][:, 128 * p:128 * (p + 1)],
                            rhs=xt_t[c][:],
                            start=(c == 0), stop=(c == 1),
                        )
                    t = wpool.tile([128, N], XDT, tag=f"qk{p}")
                    if p == 0:
                        nc.scalar.copy(t[:], ps[:, 0:N])
                    else:
                        nc.vector.tensor_copy(t[:], ps[:, 0:N])
                else:
                    for c in range(2):
                        nc.tensor.matmul(
                            ps[:, 0:kp],
                            lhsT=wqkv_sb[c][:, 128 * p:128 * (p + 1)],
                            rhs=xtk_t[c][:, 0:kp],
                            start=(c == 0), stop=(c == 1),
                        )
                    t = wpool.tile([128, KPAD], XDT, tag=f"qk{p}")
                    if p == 2:
                        nc.scalar.copy(t[:, 0:kp], ps[:, 0:kp])
                    else:
                        nc.vector.tensor_copy(t[:, 0:kp], ps[:, 0:kp])
                qk_sb.append(t)
            v_sb = []
            for j in range(nk_c):
                ps = ps1.tile([128, 512], F32, tag="b1")
                for c in range(2):
                    nc.tensor.matmul(
                        ps[0:JC, 0:DIM],
                        lhsT=xtk_t[c][:, JC * j:JC * (j + 1)],
                        rhs=wqkv_sb[c][:, 2 * DIM:3 * DIM],
                        start=(c == 0), stop=(c == 1),
                    )
                t = wpool.tile([JC, DIM], FP16, tag=f"v{j}")
                if v_eng == 'act':
                    nc.scalar.copy(t[:], ps[0:JC, 0:DIM])
                else:
                    nc.vector.tensor_copy(t[:], ps[0:JC, 0:DIM])
                v_sb.append(t)
            return qk_sb, v_sb, ebw_t, nk_c

        def emit_attn_kc(w, qk_sb, v_sb, ebw_t, nk_c):
            on_sb = []
            for hg in range(2):
                pv = pvrs.tile([128, 512], F32, tag="pvrs")
                rs = pvrs.tile([128, 512], F32, tag="pvrs")
                pts = []
                for jc in range(nk_c):
                    ptj = wpool.tile([JC, 4, N], FP16, tag=f"pt{jc}")
                    for sg in range(2):
                        smp = psim.tile([128, 1024], F32, tag="sim")
                        for i2 in range(2):
                            t4 = 2 * sg + i2
                            nc.tensor.matmul(
                                smp[0:JC, 512 * i2:512 * i2 + N],
                                lhsT=qk_sb[2 + hg][32 * t4:32 * (t4 + 1), JC * jc:JC * (jc + 1)],
                                rhs=qk_sb[hg][32 * t4:32 * (t4 + 1), :],
                                start=True, stop=True,
                                tile_position=(32 * t4, 0),
                            )
                        et = wpool.tile([JC, 2, N], FP16, tag=f"et{jc}{sg}")
                        sim_ap = smp[0:JC, :].rearrange("p (t x) -> p t x", t=2)[:, :, 0:N]
                        nc.scalar.activation(
                            et[:], sim_ap, EXP,
                            bias=msk_sb[:, NJC * w + jc:NJC * w + jc + 1],
                        )
                        eb_ap = ebw_t[:, 4 * hg + 2 * sg:4 * hg + 2 * sg + 2, jc, :]
                        nc.vector.tensor_mul(ptj[:, 2 * sg:2 * sg + 2, :], et[:], eb_ap)
                    pts.append(ptj)
                for t4 in range(4):
                    h = 4 * hg + t4
                    for jc in range(nk_c):
                        nc.tensor.matmul(
                            pv[32 * t4:32 * (t4 + 1), 0:N],
                            lhsT=v_sb[jc][:, 32 * h:32 * (h + 1)],
                            rhs=pts[jc][:, t4, :],
                            start=(jc == 0), stop=(jc == nk_c - 1),
                            tile_position=(0, 32 * t4),
                            skip_group_check=True,
                        )
                    for jc in range(nk_c):
                        nc.tensor.matmul(
                            rs[32 * t4:32 * (t4 + 1), 0:N],
                            lhsT=ones_sb[:],
                            rhs=pts[jc][:, t4, :],
                            start=(jc == 0), stop=(jc == nk_c - 1),
                            tile_position=(0, 32 * t4),
                            skip_group_check=True,
                        )
                rr = wpool.tile([128, N], F32, tag="rr")
                nc.vector.reciprocal_approx_fast(rr[:], rs[:, 0:N])
                on = wpool.tile([128, N], XDT, tag=f"on{hg}")
                nc.vector.tensor_mul(on[:], pv[:, 0:N], rr[:])
                on_sb.append(on)

            for ic in range(NJC):
                po = ps1.tile([128, 512], F32, tag="b1")
                for kc in range(2):
                    nc.tensor.matmul(
                        po[0:JC, 0:DIM],
                        lhsT=on_sb[kc][:, JC * ic:JC * (ic + 1)],
                        rhs=wout_sb[kc][:],
                        start=(kc == 0), stop=(kc == 1),
                    )
                fo = wpool.tile([JC, DIM], FP16 if out16 else F32, tag=f"fo{ic}")
                if fo_eng == 'dve':
                    nc.vector.tensor_copy(fo[:], po[0:JC, 0:DIM])
                else:
                    nc.scalar.copy(fo[:], po[0:JC, 0:DIM])
                nc.sync.dma_start(out_d[w, JC * ic:JC * (ic + 1), :], fo[:])
                if timing and w == 0:
                    nc.sync.dma_start(outx_d[0, JC * ic:JC * (ic + 1), :], fo[:])

        if augv:
            emit_attn = emit_attn_augv
        if kcomp:
            emit_qkv = emit_qkv_kc
            emit_attn = emit_attn_kc

        rep_ctx = tc.For_i(0, reps, 1) if reps > 1 else None
        if rep_ctx is not None:
            ctx.enter_context(rep_ctx)
        if pipe:
            # software pipeline: QKV of windows w+1..w+pipe emitted (and thus
            # prioritized) ahead of attention of window w, so the PE fills
            # softmax-chain stalls with upcoming windows' projections
            depth = int(pipe)
            inflight = []
            for w in range(n_wins):
                inflight.append(emit_qkv(w))
                if len(inflight) > depth:
                    emit_attn(w - depth, *inflight.pop(0))
            for i, qv in enumerate(inflight):
                emit_attn(n_wins - len(inflight) + i, *qv)
        else:
            for w in range(n_wins):
                r = emit_qkv(w)
                emit_attn(w, *r)

    nc.compile()
    return nc


NKC_SLOTS = None    # per-window-slot key-chunk counts (set by host_prep when kcomp)


def host_prep(x, mask, w_qkv, w_out, bias_table, f32r=None, kcomp=None):
    """Build per-core input maps (numpy only)."""
    global NKC_SLOTS
    if f32r is None:
        f32r = bool(BEST_CFG.get("f32r", False))
    if kcomp is None:
        kcomp = bool(BEST_CFG.get("kcomp", False))
    xdt = np.float32 if f32r else np.float16
    x = np.asarray(x, dtype=np.float32)
    mask = np.asarray(mask)
    w_qkv = np.asarray(w_qkv, dtype=np.float32)
    w_out = np.asarray(w_out, dtype=np.float32)
    bias_table = np.asarray(bias_table, dtype=np.float32)

    # x: (B, L, X, Y, W1, W2, C) -> windows (B,X,Y) x [C, N]
    xr = np.ascontiguousarray(x.transpose(0, 2, 3, 1, 4, 5, 6)).reshape(NWIN, N, DIM)
    xt = np.ascontiguousarray(xr.transpose(0, 2, 1)).reshape(NWIN, 2, 128, N).astype(xdt)

    # mask: (B, X, Y, W1, W2, 1, L) -> (B,X,Y) x N with token order (l, w1, w2)
    m = np.ascontiguousarray(mask.transpose(0, 1, 2, 5, 6, 3, 4)).reshape(NWIN, N)
    maskadd = np.where(m == 0, np.float32(MASK_NEG), np.float32(0.0)).astype(np.float32)

    # exp(bias) transposed: ebT[h, j, i] = exp(bias[i, j, h])
    ri = _rel_pos_index(AGENT, WIN, WIN)
    bias = bias_table[ri]                       # (N, N, H) [i, j, h]
    ebT = np.exp(bias.transpose(2, 1, 0))       # (H, j, i)
    eb_host = np.ascontiguousarray(
        ebT.reshape(HEADS, NJC, JC, N).transpose(2, 0, 1, 3)
    ).astype(np.float16)                # (JC, H, NJC, N)

    wq = w_qkv.copy()
    wq[0:DIM] *= np.float32(SCALE)
    wqkvT = np.ascontiguousarray(wq.T).reshape(2, 128, 3 * DIM).astype(xdt)
    woutT = np.ascontiguousarray(w_out.T).reshape(2, 128, DIM).astype(xdt)

    # sparse head layout for augv: tile T rows 0-31 = head 2T, rows 64-95 =
    # head 2T+1, other rows zero (multiplied by on-rows that are zero)
    wt = np.ascontiguousarray(w_out.T)          # [e, c']
    wout4 = np.zeros((4, 128, DIM), np.float32)
    for t in range(4):
        wout4[t, 0:32] = wt[64 * t:64 * t + 32]
        wout4[t, 64:96] = wt[64 * t + 32:64 * t + 64]
    wout4 = wout4.astype(np.float16)

    kc = {}
    if kcomp:
        # per-window key compaction: unmasked tokens first, masked as pad.
        # all 8 cores run one SPMD instruction stream, so the chunk count per
        # window SLOT is the max over cores of that slot's ceil(nk/98)
        m01 = np.ascontiguousarray(mask.transpose(0, 1, 2, 5, 6, 3, 4)).reshape(NWIN, N)
        nk = m01.sum(1)
        nkc_g = np.maximum(1, -(-nk // JC))
        nkc_slots = tuple(int(max(nkc_g[WPC * c + w] for c in range(NCORES)))
                          for w in range(WPC))
        NKC_SLOTS = nkc_slots
        nkmax = max(nkc_slots)
        kpad = JC * nkmax
        ri = _rel_pos_index(AGENT, WIN, WIN)
        ebT = np.exp(bias_table[ri].transpose(2, 1, 0)).astype(np.float32)  # (H, j, i)
        xtk = np.zeros((NWIN, 2, 128, kpad), xdt)
        ebw = np.zeros((NWIN, JC, HEADS, nkmax, N), np.float16)
        mskb = np.full((NWIN, NJC, JC), np.float32(MASK_NEG), np.float32)
        for g in range(NWIN):
            idx = np.concatenate([np.flatnonzero(m01[g] == 1), np.flatnonzero(m01[g] == 0)])[:kpad]
            xtk[g] = xr[g].T[:, idx].reshape(2, 128, kpad).astype(xdt)
            # (H, kpad, N) -> (JC, H, nkmax, N)
            ebg = ebT[:, idx, :].reshape(HEADS, nkmax, JC, N).transpose(2, 0, 1, 3)
            ebw[g] = ebg.astype(np.float16)
            pos = np.arange(kpad)
            mb = np.where(pos < nk[g], np.float32(0.0), np.float32(MASK_NEG))
            mskb[g, 0:nkmax] = mb.reshape(nkmax, JC)
        kc = {"xtk": xtk, "ebw": ebw, "mskb": mskb}

    in_maps = []
    for core in range(NCORES):
        ws = slice(WPC * core, WPC * (core + 1))
        if kcomp:
            mm = kc["mskb"][ws].transpose(2, 0, 1).reshape(JC, WPC * NJC)
        else:
            mm = maskadd[ws].reshape(WPC, NJC, JC).transpose(2, 0, 1).reshape(JC, WPC * NJC)
        im = {
            "xt": np.ascontiguousarray(xt[ws]),
            "msk": np.ascontiguousarray(mm),
            "eb": eb_host,
            "wqkv": wqkvT,
            "wout": woutT,
            "wout4": wout4,
        }
        if kcomp:
            im["xtk"] = np.ascontiguousarray(kc["xtk"][ws])
            im["ebw"] = np.ascontiguousarray(kc["ebw"][ws])
        in_maps.append(im)
    return in_maps


def assemble_output(core_outs):
    """core_outs: list of [WPC, N, DIM] arrays -> full (B, L, X, Y, W1, W2, C)."""
    out = np.concatenate([np.asarray(o) for o in core_outs], axis=0).astype(np.float32)
    out = out.reshape(B, X, Y, AGENT, WIN, WIN, DIM)
    return np.ascontiguousarray(out.transpose(0, 3, 1, 2, 4, 5, 6)).astype(np.float32)


_NC_CACHE = {}


def _get_nc(n_wins=WPC):
    key = (n_wins, NKC_SLOTS if BEST_CFG.get("kcomp") else None)
    if key not in _NC_CACHE:
        _NC_CACHE[key] = build_graph(n_wins, nkc=NKC_SLOTS, **BEST_CFG)
    return _NC_CACHE[key]


BEST_CFG = dict(wbufs=4, v_eng="dve", out16=True, sim_pairs=True, fo_eng="act", pipe=True,
                kcomp=True)


def kernel(x, mask, w_qkv, w_out, bias_table):
    in_maps = host_prep(x, mask, w_qkv, w_out, bias_table)
    nc = _get_nc(WPC)
    res = run_bass_kernel_spmd(nc, in_maps, core_ids=list(range(NCORES)))
    core_outs = [res.results[i]["out"] for i in range(NCORES)]
    return assemble_output(core_outs)



# revision 42
# speedup vs baseline: 1.3807x; 1.1188x over previous
"""Trainium2 Bass kernel for windowed sparse attention (nn_Attention_17703855194428).

Reference computation (per window w of 128 = B*X*Y, tokens N=294 = L*W1*W2):
    qkv = x_w @ w_qkv.T ; q,k,v heads (8 heads x 32 dim), q scaled by 1/sqrt(32)
    sim = q @ k.T + rel_pos_bias ; masked cols -> -1e9 ; softmax over keys
    out = (attn @ v) @ w_out.T

Sharding: pure data parallel over the 128 independent windows -> 16 windows
per NeuronCore, weights/bias replicated. No collectives.

kcomp (key compaction, BEST_CFG default): ~50% of keys are masked per
window (mask ~ Bernoulli(0.5) over the 294 key slots), and masked keys
contribute exactly exp(-1e9)=0. The host reorders each window's keys
(unmasked first, masked as pad) so the live keys fit in ceil(nk/98)=2
chunks instead of 3: sim/PV/rowsum matmuls drop 24->16 each, exp 12->8,
eb-muls 12->8, v-proj 6->4 per window (92 -> 66 matmuls total, -30% PE
stream, -33% ACT/DVE). Queries keep all 294 tokens and original order, so
the output layout is unchanged. Per-window compacted exp-bias tables
(ebw, 922KB/window) are streamed per window through a 3-deep SBUF ring
instead of one shared table; the additive mask bias now encodes compacted
position >= nk (pad) rather than the original mask. All 8 cores run one
SPMD instruction stream, so each window slot's chunk count is the max
over cores (still 2 for this input distribution, nk ~ 147 +- 9).

Kernel layout strategy (per core, per window; all matmul inputs fp16,
fp32 PSUM accumulate; host pre-computes exp(rel-pos-bias) transposed and the
additive mask in the kernel's SBUF layouts):
    xT [C=256, N=294] (channels on partitions) ->
    q,k as [E, N] (head-dim on partitions), v as [N, E] (tokens on partitions)
    simT[j, i] = sum_d k[d,j] q[d,i] via row-tiled (K=32) matmuls, 2 heads
        per 2-bank psum tile, double-buffered (sim_pairs)
    P_T = exp(simT + mask_j) * exp(bias)_T   (one ACT exp with per-partition
        mask bias reading across psum banks + one DVE fp16 multiply)
    outU.T[hd, i] = sum_j v[j, hd] P_T[j, i] via col-tiled (M=32) matmuls
    rowsum broadcast to all 32 head rows for free via col-tiled ones-matmul
    1/rowsum via DVE reciprocal_approx_fast; normalize is one DVE multiply
    out = (outU.T * (1/rowsum)).T @ w_out.T via K=hd matmuls -> [tok, C],
    written fp16 (host upcasts to fp32)

Measured on HW (differential For_i timing): 319-410 us per core for 16
windows for THIS config across one session — run-to-run variance on the
shared axon TRN2 is large (up to ~25%), so single-run deltas under ~10% are
not trustworthy; 1-stage software-pipelined emission (QKV of window w+1
ahead of attn of w). PE is the bottleneck: this toolchain
serializes every Ldweights+Matmult pair (--enable-ldw-opt=false, per-MM sem
updates), ~15.5 us/window of serial PE.

Optimization attempts measured SLOWER than this config (kept as flags):
  rs_presum=True   364 us: DVE pt-chunk adds sit on the softmax critical path
  pipe=2           345 us: deeper pipeline thrashes the 2-slot ps1 psum ring
  f32r=True        372 us: f32r (self-loading-weights) qkv/sim/outproj MMs
                   are slower than fp16+Ldweights pairs despite the cost
                   model's >=256-free fast path; numerics better (5.3e-4)
  augv=True        (broken + net-negative): fused rowsum-in-PV via augmented
                   v columns; engine ops cannot read/write different base
                   partitions (verified NaN on HW), so the 1/rowsum
                   broadcast back to head rows needs extra PE/DVE work that
                   cancels the 24 saved ones-matmuls
HW facts verified by probes this session: tile-pool ring slots do keep
warmup-written bytes across later same-tag allocations; DVE/ACT ops with
in/out base-partition mismatch return garbage; DMA cannot read PSUM;
matmul operand base partitions must be equal and in {0,32,64}; engine
partition accesses must start at 0/32/64/96; f32r matmul inputs must be
produced rounded (F32R-dtype outputs of ACT/DVE copies or DMA loads work).
"""

import numpy as np
from contextlib import ExitStack

import concourse.bass as bass
import concourse.bacc as bacc
import concourse.mybir as mybir
from concourse import tile
from concourse.bass_utils import run_bass_kernel_spmd

import ml_dtypes

F32 = mybir.dt.float32
F32R = mybir.dt.float32r
BF16 = mybir.dt.bfloat16
FP16 = mybir.dt.float16
EXP = mybir.ActivationFunctionType.Exp

# Problem constants (hardcoded per harness contract)
B, AGENT, X, Y, WIN, DIM, HEADS, DH = 2, 6, 8, 8, 7, 256, 8, 32
N = AGENT * WIN * WIN            # 294 tokens per window
NWIN = B * X * Y                 # 128 windows
NCORES = 8
WPC = NWIN // NCORES             # 16 windows per core
JC = 98                          # key-chunk size (294 = 3*98)
NJC = 3
SCALE = DH ** -0.5
MASK_NEG = -1e9


def _rel_pos_index(L, Wh, Ww):
    coords = np.stack(np.meshgrid(np.arange(L), np.arange(Wh), np.arange(Ww), indexing="ij"))
    cf = coords.reshape(3, -1)
    rel = cf[:, :, None] - cf[:, None, :]
    rel = rel.transpose(1, 2, 0).astype(np.int64)
    rel[..., 0] += L - 1
    rel[..., 1] += Wh - 1
    rel[..., 2] += Ww - 1
    rel[..., 0] *= (2 * Wh - 1) * (2 * Ww - 1)
    rel[..., 1] *= 2 * Ww - 1
    return rel.sum(-1)  # (N, N) [i, j]


def build_graph(n_wins=WPC, wbufs=2, pvrs_bufs=2, misc_bufs=2, trace_sim=False, reps=1,
                v_eng='act', fo_eng='dve', gp_jc=(), sim_pairs=False, timing=False, out16=False, ab_noexp=False, ab_nosim=False, ab_nopv=False, pipe=False, sim_bufs=2, rs_presum=False,
                augv=False, rrb_sb=False, recip_eng='dve', f32r=False,
                kcomp=False, nkc=None, rs_first=True):
    nc = bacc.Bacc(None)
    XDT = F32R if f32r else FP16
    nkc_l = list(nkc) if nkc else [NJC] * n_wins
    NKMAX = max(nkc_l)
    KPAD = JC * NKMAX
    n_xt = 2 if timing else n_wins
    xt_d = nc.declare_dram_parameter("xt", [n_xt, 2, 128, N], XDT, isOutput=False)
    if kcomp:
        # mask-compacted keys: per window only ceil(nk/98) chunks of keys
        # (unmasked tokens first, masked-as-pad after) take part in
        # sim/softmax/PV/rowsum; queries stay all N tokens
        xtk_d = nc.declare_dram_parameter("xtk", [n_xt, 2, 128, KPAD], XDT, isOutput=False)
        ebw_d = nc.declare_dram_parameter("ebw", [n_xt, JC, HEADS, NKMAX, N], FP16, isOutput=False)
    msk_d = nc.declare_dram_parameter("msk", [JC, n_wins * NJC], F32, isOutput=False)
    eb_d = None
    if not kcomp:
        eb_d = nc.declare_dram_parameter("eb", [JC, HEADS, NJC, N], FP16, isOutput=False)
    wqkv_d = nc.declare_dram_parameter("wqkv", [2, 128, 3 * DIM], XDT, isOutput=False)
    if augv:
        wout_d = nc.declare_dram_parameter("wout4", [4, 128, DIM], FP16, isOutput=False)
    else:
        wout_d = nc.declare_dram_parameter("wout", [2, 128, DIM], XDT, isOutput=False)
    ODT = FP16 if out16 else F32
    if timing:
        out_d = nc.dram_tensor("oscr", [n_wins, N, DIM], ODT)
        outx_d = nc.declare_dram_parameter("out", [1, N, DIM], ODT, isOutput=True)
    else:
        out_d = nc.declare_dram_parameter("out", [n_wins, N, DIM], ODT, isOutput=True)
        outx_d = None

    with tile.TileContext(nc, trace_sim=trace_sim) as tc, ExitStack() as ctx:
        cpool = ctx.enter_context(tc.tile_pool(name="consts", bufs=1))
        wpool = ctx.enter_context(tc.tile_pool(name="work", bufs=wbufs))
        # one xt slot per window: slot reuse on DMA-written tiles piles up
        # sync waits beyond what DMA descriptors support
        xpool = ctx.enter_context(tc.tile_pool(name="xin", bufs=n_wins))
        if sim_pairs:
            psim = ctx.enter_context(tc.tile_pool(name="psim", bufs=sim_bufs, space="PSUM"))
        else:
            psim = ctx.enter_context(tc.tile_pool(name="psim", bufs=1, space="PSUM"))
        pvrs = ctx.enter_context(tc.tile_pool(name="pvrs", bufs=pvrs_bufs, space="PSUM"))
        ps1 = ctx.enter_context(tc.tile_pool(name="ps1", bufs=misc_bufs, space="PSUM"))
        if kcomp:
            ebpool = ctx.enter_context(tc.tile_pool(name="ebp", bufs=3))

        # ---- replicated constants ----
        wqkv_sb = []
        for c in range(2):
            t = cpool.tile([128, 3 * DIM], XDT, tag=f"wqkv{c}")
            nc.sync.dma_start(t[:], wqkv_d[c])
            wqkv_sb.append(t)
        wout_sb = []
        for c in range(4 if augv else 2):
            t = cpool.tile([128, DIM], FP16 if augv else XDT, tag=f"wout{c}")
            nc.sync.dma_start(t[:], wout_d[c])
            wout_sb.append(t)

        if not kcomp:
            eb_sb = cpool.tile([JC, HEADS, NJC, N], FP16, tag="eb")
            nc.sync.dma_start(eb_sb[:], eb_d[:])
        msk_sb = cpool.tile([JC, n_wins * NJC], F32, tag="msk")
        nc.sync.dma_start(msk_sb[:], msk_d[:])
        ones_sb = cpool.tile([JC, 32], FP16, tag="ones")
        nc.vector.memset(ones_sb[:], 1.0)

        if ab_noexp or ab_nosim:
            pt_const = cpool.tile([JC, 4, N], FP16, tag="ptc")
            nc.vector.memset(pt_const[:], 0.01)
            stub = cpool.tile([1, 8], F32, tag="stub")

        # warm-up touches: absorb the one-time const-DMA waits into throwaway
        # instructions so steady-state ops stay within the per-instruction
        # sync-wait budget
        scr_a = cpool.tile([JC, 1], F32, tag="scr_a")
        nc.scalar.copy(scr_a[:], msk_sb[:, 0:1])
        scr_v = cpool.tile([JC, 1], FP16, tag="scr_v")
        if kcomp:
            nc.vector.tensor_copy(scr_v[:], wqkv_sb[0][0:JC, 0:1])
        else:
            nc.vector.tensor_copy(scr_v[:], eb_sb[:, 0, 0, 0:1])

        if augv:
            # one-time init of the v-tile ring slots: cols 32-63 of each head
            # block stay 1.0 forever (per-window copies write only cols 0-31),
            # making the PV matmul's output rows 32-63 the rowsum replicated
            # 32x (free broadcast for the normalize)
            for j in range(NJC):
                for _ in range(wbufs):
                    t = wpool.tile([JC, HEADS, 2 * DH], FP16, tag=f"v{j}")
                    nc.vector.memset(t[:, :, DH:2 * DH], 1.0)
            # one-time zero of the on-tile ring slots' unused rows (32-63,
            # 96-127): they multiply wout4's zero rows in the out-projection
            # and must be finite; per-window muls only write the head rows
            for tg in range(4):
                for _ in range(wbufs):
                    t = wpool.tile([128, N], FP16, tag=f"on{tg}")
                    nc.vector.memset(t[32:64, :], 0.0)
                    nc.vector.memset(t[96:128, :], 0.0)

        def emit_qkv(w):
            xt_t = []
            for c in range(2):
                t = xpool.tile([128, N], XDT, tag=f"xt{c}")
                nc.sync.dma_start(t[:], xt_d[w % n_xt, c])
                xt_t.append(t)
            qk_sb = []
            for p in range(4):
                ps = ps1.tile([128, 512], F32, tag="b1")
                for c in range(2):
                    nc.tensor.matmul(
                        ps[:, 0:N],
                        lhsT=wqkv_sb[c][:, 128 * p:128 * (p + 1)],
                        rhs=xt_t[c][:],
                        start=(c == 0), stop=(c == 1),
                    )
                t = wpool.tile([128, N], XDT, tag=f"qk{p}")
                if p < 2:
                    nc.scalar.copy(t[:], ps[:, 0:N])
                else:
                    nc.vector.tensor_copy(t[:], ps[:, 0:N])
                qk_sb.append(t)
            v_sb = []
            for j in range(NJC):
                ps = ps1.tile([128, 512], F32, tag="b1")
                for c in range(2):
                    nc.tensor.matmul(
                        ps[0:JC, 0:DIM],
                        lhsT=xt_t[c][:, JC * j:JC * (j + 1)],
                        rhs=wqkv_sb[c][:, 2 * DIM:3 * DIM],
                        start=(c == 0), stop=(c == 1),
                    )
                if augv:
                    # [j, head, 64]: cols 0-31 = v channels, cols 32-63 = 1.0
                    # (pre-set once per ring slot) so PV output rows 32-63 are
                    # the softmax rowsum replicated across 32 partitions
                    t = wpool.tile([JC, HEADS, 2 * DH], FP16, tag=f"v{j}")
                    src = ps[0:JC, 0:DIM].rearrange("p (h e) -> p h e", h=HEADS)
                    if v_eng == 'act':
                        nc.scalar.copy(t[:, :, 0:DH], src)
                    else:
                        nc.vector.tensor_copy(t[:, :, 0:DH], src)
                else:
                    t = wpool.tile([JC, DIM], FP16, tag=f"v{j}")
                    if v_eng == 'act':
                        nc.scalar.copy(t[:], ps[0:JC, 0:DIM])
                    else:
                        nc.vector.tensor_copy(t[:], ps[0:JC, 0:DIM])
                v_sb.append(t)
            return qk_sb, v_sb

        def emit_attn(w, qk_sb, v_sb):
            on_sb = []
            for hg in range(2):
                pv = pvrs.tile([128, 512], F32, tag="pvrs")
                rs = pvrs.tile([128, 512], F32, tag="pvrs")
                pts = []
                if ab_nosim:
                    pts = [pt_const] * NJC
                elif sim_pairs:
                    for jc in range(NJC):
                        ptj = wpool.tile([JC, 4, N], FP16, tag=f"pt{jc}")
                        for sg in range(2):
                            smp = psim.tile([128, 1024], F32, tag="sim")
                            for i2 in range(2):
                                t4 = 2 * sg + i2
                                nc.tensor.matmul(
                                    smp[0:JC, 512 * i2:512 * i2 + N],
                                    lhsT=qk_sb[2 + hg][32 * t4:32 * (t4 + 1), JC * jc:JC * (jc + 1)],
                                    rhs=qk_sb[hg][32 * t4:32 * (t4 + 1), :],
                                    start=True, stop=True,
                                    tile_position=(32 * t4, 0),
                                )
                            et = wpool.tile([JC, 2, N], FP16, tag=f"et{jc}{sg}")
                            sim_ap = smp[0:JC, :].rearrange("p (t x) -> p t x", t=2)[:, :, 0:N]
                            nc.scalar.activation(
                                et[:], sim_ap, EXP,
                                bias=msk_sb[:, NJC * w + jc:NJC * w + jc + 1],
                            )
                            eb_ap = eb_sb[:, 4 * hg + 2 * sg:4 * hg + 2 * sg + 2, jc, :]
                            nc.vector.tensor_mul(ptj[:, 2 * sg:2 * sg + 2, :], et[:], eb_ap)
                        pts.append(ptj)
                else:
                    for jc in range(NJC):
                        smp = psim.tile([128, 2048], F32, tag="sim")
                        for t4 in range(4):
                            nc.tensor.matmul(
                                smp[0:JC, 512 * t4:512 * t4 + N],
                                lhsT=qk_sb[2 + hg][32 * t4:32 * (t4 + 1), JC * jc:JC * (jc + 1)],
                                rhs=qk_sb[hg][32 * t4:32 * (t4 + 1), :],
                                start=True, stop=True,
                                tile_position=(32 * t4, 0),
                            )
                        if ab_noexp:
                            nc.scalar.copy(stub[:], smp[0:1, 0:8])
                            pts.append(pt_const)
                            continue
                        et = wpool.tile([JC, 4, N], FP16, tag=f"et{jc}")
                        sim_ap = smp[0:JC, :].rearrange("p (t x) -> p t x", t=4)[:, :, 0:N]
                        nc.scalar.activation(
                            et[:], sim_ap, EXP,
                            bias=msk_sb[:, NJC * w + jc:NJC * w + jc + 1],
                        )
                        pt = wpool.tile([JC, 4, N], FP16, tag=f"pt{jc}")
                        eb_ap = eb_sb[:, 4 * hg:4 * (hg + 1), jc, :]
                        nc.vector.tensor_mul(pt[:], et[:], eb_ap)
                        pts.append(pt)

                ptsum = None
                if rs_presum and not (ab_nopv or ab_nosim or ab_noexp):
                    # rowsum needs sum over all 294 j; sum the three j-chunks
                    # on DVE first so each head needs one ones-matmul, not 3
                    ptsum = wpool.tile([JC, 4, N], FP16, tag="ptsum")
                    nc.vector.tensor_add(ptsum[:], pts[0][:], pts[1][:])
                    nc.vector.tensor_add(ptsum[:], ptsum[:], pts[2][:])
                pv_iters = [(0, [0])] if ab_nopv else [(t, list(range(NJC))) for t in range(4)]
                for t4, jcs in pv_iters:
                    h = 4 * hg + t4
                    for jc in jcs:
                        nc.tensor.matmul(
                            pv[32 * t4:32 * (t4 + 1), 0:N],
                            lhsT=v_sb[jc][:, 32 * h:32 * (h + 1)],
                            rhs=pts[jc][:, t4, :],
                            start=(jc == 0), stop=(jc == jcs[-1]),
                            tile_position=(0, 32 * t4),
                            skip_group_check=True,
                        )
                    if ptsum is not None:
                        nc.tensor.matmul(
                            rs[32 * t4:32 * (t4 + 1), 0:N],
                            lhsT=ones_sb[:],
                            rhs=ptsum[:, t4, :],
                            start=True, stop=True,
                            tile_position=(0, 32 * t4),
                            skip_group_check=True,
                        )
                        continue
                    for jc in jcs:
                        nc.tensor.matmul(
                            rs[32 * t4:32 * (t4 + 1), 0:N],
                            lhsT=ones_sb[:],
                            rhs=pts[jc][:, t4, :],
                            start=(jc == 0), stop=(jc == jcs[-1]),
                            tile_position=(0, 32 * t4),
                            skip_group_check=True,
                        )
                rr = wpool.tile([128, N], F32, tag="rr")
                nc.vector.reciprocal_approx_fast(rr[:], rs[:, 0:N])
                on = wpool.tile([128, N], XDT, tag=f"on{hg}")
                nc.vector.tensor_mul(on[:], pv[:, 0:N], rr[:])
                on_sb.append(on)

            for ic in range(NJC):
                po = ps1.tile([128, 512], F32, tag="b1")
                for kc in range(2):
                    nc.tensor.matmul(
                        po[0:JC, 0:DIM],
                        lhsT=on_sb[kc][:, JC * ic:JC * (ic + 1)],
                        rhs=wout_sb[kc][:],
                        start=(kc == 0), stop=(kc == 1),
                    )
                fo = wpool.tile([JC, DIM], FP16 if out16 else F32, tag=f"fo{ic}")
                if fo_eng == 'dve':
                    nc.vector.tensor_copy(fo[:], po[0:JC, 0:DIM])
                else:
                    nc.scalar.copy(fo[:], po[0:JC, 0:DIM])
                nc.sync.dma_start(out_d[w, JC * ic:JC * (ic + 1), :], fo[:])
                if timing and w == 0:
                    nc.sync.dma_start(outx_d[0, JC * ic:JC * (ic + 1), :], fo[:])

        def emit_attn_augv(w, qk_sb, v_sb):
            """PV matmuls also produce the softmax rowsum (augmented-v 33rd
            column); no ones-matmul rowsum pass. Head pair per psum tile at
            col positions 0/64; rowsums land on partitions 32/96."""
            on_sb = [None] * 4
            for hg in range(2):
                pts = []
                for jc in range(NJC):
                    ptj = wpool.tile([JC, 4, N], FP16, tag=f"pt{jc}")
                    for sg in range(2):
                        smp = psim.tile([128, 1024], F32, tag="sim")
                        for i2 in range(2):
                            t4 = 2 * sg + i2
                            nc.tensor.matmul(
                                smp[0:JC, 512 * i2:512 * i2 + N],
                                lhsT=qk_sb[2 + hg][32 * t4:32 * (t4 + 1), JC * jc:JC * (jc + 1)],
                                rhs=qk_sb[hg][32 * t4:32 * (t4 + 1), :],
                                start=True, stop=True,
                                tile_position=(32 * t4, 0),
                            )
                        et = wpool.tile([JC, 2, N], FP16, tag=f"et{jc}{sg}")
                        sim_ap = smp[0:JC, :].rearrange("p (t x) -> p t x", t=2)[:, :, 0:N]
                        nc.scalar.activation(
                            et[:], sim_ap, EXP,
                            bias=msk_sb[:, NJC * w + jc:NJC * w + jc + 1],
                        )
                        eb_ap = eb_sb[:, 4 * hg + 2 * sg:4 * hg + 2 * sg + 2, jc, :]
                        nc.vector.tensor_mul(ptj[:, 2 * sg:2 * sg + 2, :], et[:], eb_ap)
                    pts.append(ptj)

                for tl in range(2):
                    pv = pvrs.tile([128, 512], F32, tag="pvrs")
                    for hp in range(2):
                        hl = 2 * tl + hp
                        hglob = 4 * hg + hl
                        for jc in range(NJC):
                            nc.tensor.matmul(
                                pv[64 * hp:64 * (hp + 1), 0:N],
                                lhsT=v_sb[jc][:, hglob, :],
                                rhs=pts[jc][:, hl, :],
                                start=(jc == 0), stop=(jc == NJC - 1),
                                tile_position=(0, 64 * hp),
                                skip_group_check=True,
                            )
                    # pv rows: 0-31 headA out, 32-63 headA rowsum (x32),
                    #          64-95 headB out, 96-127 headB rowsum (x32)
                    tg = 2 * hg + tl
                    rb = wpool.tile([96, N], F32, tag=f"rb{tg}")
                    nc.vector.reciprocal_approx_fast(rb[0:32, :], pv[32:64, 0:N])
                    nc.vector.reciprocal_approx_fast(rb[64:96, :], pv[96:128, 0:N])
                    on = wpool.tile([128, N], FP16, tag=f"on{tg}")
                    nc.vector.tensor_mul(on[0:32, :], pv[0:32, 0:N], rb[0:32, :])
                    nc.vector.tensor_mul(on[64:96, :], pv[64:96, 0:N], rb[64:96, :])
                    on_sb[tg] = on

            for ic in range(NJC):
                po = ps1.tile([128, 512], F32, tag="b1")
                for tg in range(4):
                    nc.tensor.matmul(
                        po[0:JC, 0:DIM],
                        lhsT=on_sb[tg][:, JC * ic:JC * (ic + 1)],
                        rhs=wout_sb[tg][:],
                        start=(tg == 0), stop=(tg == 3),
                    )
                fo = wpool.tile([JC, DIM], FP16 if out16 else F32, tag=f"fo{ic}")
                if fo_eng == 'dve':
                    nc.vector.tensor_copy(fo[:], po[0:JC, 0:DIM])
                else:
                    nc.scalar.copy(fo[:], po[0:JC, 0:DIM])
                nc.sync.dma_start(out_d[w, JC * ic:JC * (ic + 1), :], fo[:])
                if timing and w == 0:
                    nc.sync.dma_start(outx_d[0, JC * ic:JC * (ic + 1), :], fo[:])

        def emit_qkv_kc(w):
            nk_c = nkc_l[w]
            kp = JC * nk_c
            xt_t = []
            for c in range(2):
                t = xpool.tile([128, N], XDT, tag=f"xt{c}")
                nc.sync.dma_start(t[:], xt_d[w % n_xt, c])
                xt_t.append(t)
            xtk_t = []
            for c in range(2):
                t = xpool.tile([128, KPAD], XDT, tag=f"xtk{c}")
                nc.sync.dma_start(t[:, 0:kp], xtk_d[w % n_xt, c, :, 0:kp])
                xtk_t.append(t)
            ebw_t = ebpool.tile([JC, HEADS, NKMAX, N], FP16, tag="ebw")
            nc.sync.dma_start(ebw_t[:, :, 0:nk_c, :], ebw_d[w % n_xt, :, :, 0:nk_c, :])
            qk_sb = []
            for p in range(4):
                ps = ps1.tile([128, 512], F32, tag="b1")
                if p < 2:
                    for c in range(2):
                        nc.tensor.matmul(
                            ps[:, 0:N],
                            lhsT=wqkv_sb[c][:, 128 * p:128 * (p + 1)],
                            rhs=xt_t[c][:],
                            start=(c == 0), stop=(c == 1),
                        )
                    t = wpool.tile([128, N], XDT, tag=f"qk{p}")
                    if p == 0:
                        nc.scalar.copy(t[:], ps[:, 0:N])
                    else:
                        nc.vector.tensor_copy(t[:], ps[:, 0:N])
                else:
                    for c in range(2):
                        nc.tensor.matmul(
                            ps[:, 0:kp],
                            lhsT=wqkv_sb[c][:, 128 * p:128 * (p + 1)],
                            rhs=xtk_t[c][:, 0:kp],
                            start=(c == 0), stop=(c == 1),
                        )
                    t = wpool.tile([128, KPAD], XDT, tag=f"qk{p}")
                    if p == 2:
                        nc.scalar.copy(t[:, 0:kp], ps[:, 0:kp])
                    else:
                        nc.vector.tensor_copy(t[:, 0:kp], ps[:, 0:kp])
                qk_sb.append(t)
            v_sb = []
            for j in range(nk_c):
                ps = ps1.tile([128, 512], F32, tag="b1")
                for c in range(2):
                    nc.tensor.matmul(
                        ps[0:JC, 0:DIM],
                        lhsT=xtk_t[c][:, JC * j:JC * (j + 1)],
                        rhs=wqkv_sb[c][:, 2 * DIM:3 * DIM],
                        start=(c == 0), stop=(c == 1),
                    )
                t = wpool.tile([JC, DIM], FP16, tag=f"v{j}")
                if v_eng == 'act':
                    nc.scalar.copy(t[:], ps[0:JC, 0:DIM])
                else:
                    nc.vector.tensor_copy(t[:], ps[0:JC, 0:DIM])
                v_sb.append(t)
            return qk_sb, v_sb, ebw_t, nk_c

        def emit_attn_kc(w, qk_sb, v_sb, ebw_t, nk_c):
            on_sb = []
            for hg in range(2):
                pv = pvrs.tile([128, 512], F32, tag="pvrs")
                rs = pvrs.tile([128, 512], F32, tag="pvrs")
                pts = []
                for jc in range(nk_c):
                    ptj = wpool.tile([JC, 4, N], FP16, tag=f"pt{jc}")
                    for sg in range(2):
                        smp = psim.tile([128, 1024], F32, tag="sim")
                        for i2 in range(2):
                            t4 = 2 * sg + i2
                            nc.tensor.matmul(
                                smp[0:JC, 512 * i2:512 * i2 + N],
                                lhsT=qk_sb[2 + hg][32 * t4:32 * (t4 + 1), JC * jc:JC * (jc + 1)],
                                rhs=qk_sb[hg][32 * t4:32 * (t4 + 1), :],
                                start=True, stop=True,
                                tile_position=(32 * t4, 0),
                            )
                        et = wpool.tile([JC, 2, N], FP16, tag=f"et{jc}{sg}")
                        sim_ap = smp[0:JC, :].rearrange("p (t x) -> p t x", t=2)[:, :, 0:N]
                        nc.scalar.activation(
                            et[:], sim_ap, EXP,
                            bias=msk_sb[:, NJC * w + jc:NJC * w + jc + 1],
                        )
                        eb_ap = ebw_t[:, 4 * hg + 2 * sg:4 * hg + 2 * sg + 2, jc, :]
                        nc.vector.tensor_mul(ptj[:, 2 * sg:2 * sg + 2, :], et[:], eb_ap)
                    pts.append(ptj)
                def emit_rs():
                    for t4 in range(4):
                        for jc in range(nk_c):
                            nc.tensor.matmul(
                                rs[32 * t4:32 * (t4 + 1), 0:N],
                                lhsT=ones_sb[:],
                                rhs=pts[jc][:, t4, :],
                                start=(jc == 0), stop=(jc == nk_c - 1),
                                tile_position=(0, 32 * t4),
                                skip_group_check=True,
                            )

                def emit_pv():
                    for t4 in range(4):
                        h = 4 * hg + t4
                        for jc in range(nk_c):
                            nc.tensor.matmul(
                                pv[32 * t4:32 * (t4 + 1), 0:N],
                                lhsT=v_sb[jc][:, 32 * h:32 * (h + 1)],
                                rhs=pts[jc][:, t4, :],
                                start=(jc == 0), stop=(jc == nk_c - 1),
                                tile_position=(0, 32 * t4),
                                skip_group_check=True,
                            )

                rr = wpool.tile([128, N], F32, tag="rr")
                if rs_first:
                    # rowsum matmuls first: rs completes before the pv
                    # matmuls finish, so the DVE reciprocal overlaps the pv
                    # tail instead of serializing after it
                    emit_rs()
                    nc.vector.reciprocal_approx_fast(rr[:], rs[:, 0:N])
                    emit_pv()
                else:
                    emit_pv()
                    emit_rs()
                    nc.vector.reciprocal_approx_fast(rr[:], rs[:, 0:N])
                on = wpool.tile([128, N], XDT, tag=f"on{hg}")
                nc.vector.tensor_mul(on[:], pv[:, 0:N], rr[:])
                on_sb.append(on)

            for ic in range(NJC):
                po = ps1.tile([128, 512], F32, tag="b1")
                for kc in range(2):
                    nc.tensor.matmul(
                        po[0:JC, 0:DIM],
                        lhsT=on_sb[kc][:, JC * ic:JC * (ic + 1)],
                        rhs=wout_sb[kc][:],
                        start=(kc == 0), stop=(kc == 1),
                    )
                fo = wpool.tile([JC, DIM], FP16 if out16 else F32, tag=f"fo{ic}")
                if fo_eng == 'dve':
                    nc.vector.tensor_copy(fo[:], po[0:JC, 0:DIM])
                else:
                    nc.scalar.copy(fo[:], po[0:JC, 0:DIM])
                nc.sync.dma_start(out_d[w, JC * ic:JC * (ic + 1), :], fo[:])
                if timing and w == 0:
                    nc.sync.dma_start(outx_d[0, JC * ic:JC * (ic + 1), :], fo[:])

        if augv:
            emit_attn = emit_attn_augv
        if kcomp:
            emit_qkv = emit_qkv_kc
            emit_attn = emit_attn_kc

        rep_ctx = tc.For_i(0, reps, 1) if reps > 1 else None
        if rep_ctx is not None:
            ctx.enter_context(rep_ctx)
        if pipe:
            # software pipeline: QKV of windows w+1..w+pipe emitted (and thus
            # prioritized) ahead of attention of window w, so the PE fills
            # softmax-chain stalls with upcoming windows' projections
            depth = int(pipe)
            inflight = []
            for w in range(n_wins):
                inflight.append(emit_qkv(w))
                if len(inflight) > depth:
                    emit_attn(w - depth, *inflight.pop(0))
            for i, qv in enumerate(inflight):
                emit_attn(n_wins - len(inflight) + i, *qv)
        else:
            for w in range(n_wins):
                r = emit_qkv(w)
                emit_attn(w, *r)

    nc.compile()
    return nc


NKC_SLOTS = None    # per-window-slot key-chunk counts (set by host_prep when kcomp)


def host_prep(x, mask, w_qkv, w_out, bias_table, f32r=None, kcomp=None):
    """Build per-core input maps (numpy only)."""
    global NKC_SLOTS
    if f32r is None:
        f32r = bool(BEST_CFG.get("f32r", False))
    if kcomp is None:
        kcomp = bool(BEST_CFG.get("kcomp", False))
    xdt = np.float32 if f32r else np.float16
    x = np.asarray(x, dtype=np.float32)
    mask = np.asarray(mask)
    w_qkv = np.asarray(w_qkv, dtype=np.float32)
    w_out = np.asarray(w_out, dtype=np.float32)
    bias_table = np.asarray(bias_table, dtype=np.float32)

    # x: (B, L, X, Y, W1, W2, C) -> windows (B,X,Y) x [C, N]
    xr = np.ascontiguousarray(x.transpose(0, 2, 3, 1, 4, 5, 6)).reshape(NWIN, N, DIM)
    xt = np.ascontiguousarray(xr.transpose(0, 2, 1)).reshape(NWIN, 2, 128, N).astype(xdt)

    # mask: (B, X, Y, W1, W2, 1, L) -> (B,X,Y) x N with token order (l, w1, w2)
    m = np.ascontiguousarray(mask.transpose(0, 1, 2, 5, 6, 3, 4)).reshape(NWIN, N)
    maskadd = np.where(m == 0, np.float32(MASK_NEG), np.float32(0.0)).astype(np.float32)

    # exp(bias) transposed: ebT[h, j, i] = exp(bias[i, j, h])
    ri = _rel_pos_index(AGENT, WIN, WIN)
    bias = bias_table[ri]                       # (N, N, H) [i, j, h]
    ebT = np.exp(bias.transpose(2, 1, 0))       # (H, j, i)
    eb_host = np.ascontiguousarray(
        ebT.reshape(HEADS, NJC, JC, N).transpose(2, 0, 1, 3)
    ).astype(np.float16)                # (JC, H, NJC, N)

    wq = w_qkv.copy()
    wq[0:DIM] *= np.float32(SCALE)
    wqkvT = np.ascontiguousarray(wq.T).reshape(2, 128, 3 * DIM).astype(xdt)
    woutT = np.ascontiguousarray(w_out.T).reshape(2, 128, DIM).astype(xdt)

    # sparse head layout for augv: tile T rows 0-31 = head 2T, rows 64-95 =
    # head 2T+1, other rows zero (multiplied by on-rows that are zero)
    wt = np.ascontiguousarray(w_out.T)          # [e, c']
    wout4 = np.zeros((4, 128, DIM), np.float32)
    for t in range(4):
        wout4[t, 0:32] = wt[64 * t:64 * t + 32]
        wout4[t, 64:96] = wt[64 * t + 32:64 * t + 64]
    wout4 = wout4.astype(np.float16)

    kc = {}
    if kcomp:
        # per-window key compaction: unmasked tokens first, masked as pad.
        # all 8 cores run one SPMD instruction stream, so the chunk count per
        # window SLOT is the max over cores of that slot's ceil(nk/98)
        m01 = np.ascontiguousarray(mask.transpose(0, 1, 2, 5, 6, 3, 4)).reshape(NWIN, N)
        nk = m01.sum(1)
        nkc_g = np.maximum(1, -(-nk // JC))
        nkc_slots = tuple(int(max(nkc_g[WPC * c + w] for c in range(NCORES)))
                          for w in range(WPC))
        NKC_SLOTS = nkc_slots
        nkmax = max(nkc_slots)
        kpad = JC * nkmax
        ri = _rel_pos_index(AGENT, WIN, WIN)
        ebT = np.exp(bias_table[ri].transpose(2, 1, 0)).astype(np.float32)  # (H, j, i)
        xtk = np.zeros((NWIN, 2, 128, kpad), xdt)
        ebw = np.zeros((NWIN, JC, HEADS, nkmax, N), np.float16)
        mskb = np.full((NWIN, NJC, JC), np.float32(MASK_NEG), np.float32)
        for g in range(NWIN):
            idx = np.concatenate([np.flatnonzero(m01[g] == 1), np.flatnonzero(m01[g] == 0)])[:kpad]
            xtk[g] = xr[g].T[:, idx].reshape(2, 128, kpad).astype(xdt)
            # (H, kpad, N) -> (JC, H, nkmax, N)
            ebg = ebT[:, idx, :].reshape(HEADS, nkmax, JC, N).transpose(2, 0, 1, 3)
            ebw[g] = ebg.astype(np.float16)
            pos = np.arange(kpad)
            mb = np.where(pos < nk[g], np.float32(0.0), np.float32(MASK_NEG))
            mskb[g, 0:nkmax] = mb.reshape(nkmax, JC)
        kc = {"xtk": xtk, "ebw": ebw, "mskb": mskb}

    in_maps = []
    for core in range(NCORES):
        ws = slice(WPC * core, WPC * (core + 1))
        if kcomp:
            mm = kc["mskb"][ws].transpose(2, 0, 1).reshape(JC, WPC * NJC)
        else:
            mm = maskadd[ws].reshape(WPC, NJC, JC).transpose(2, 0, 1).reshape(JC, WPC * NJC)
        im = {
            "xt": np.ascontiguousarray(xt[ws]),
            "msk": np.ascontiguousarray(mm),
            "eb": eb_host,
            "wqkv": wqkvT,
            "wout": woutT,
            "wout4": wout4,
        }
        if kcomp:
            im["xtk"] = np.ascontiguousarray(kc["xtk"][ws])
            im["ebw"] = np.ascontiguousarray(kc["ebw"][ws])
        in_maps.append(im)
    return in_maps


def assemble_output(core_outs):
    """core_outs: list of [WPC, N, DIM] arrays -> full (B, L, X, Y, W1, W2, C)."""
    out = np.concatenate([np.asarray(o) for o in core_outs], axis=0).astype(np.float32)
    out = out.reshape(B, X, Y, AGENT, WIN, WIN, DIM)
    return np.ascontiguousarray(out.transpose(0, 3, 1, 2, 4, 5, 6)).astype(np.float32)


_NC_CACHE = {}


def _get_nc(n_wins=WPC):
    key = (n_wins, NKC_SLOTS if BEST_CFG.get("kcomp") else None)
    if key not in _NC_CACHE:
        _NC_CACHE[key] = build_graph(n_wins, nkc=NKC_SLOTS, **BEST_CFG)
    return _NC_CACHE[key]


BEST_CFG = dict(wbufs=4, v_eng="dve", out16=True, sim_pairs=True, fo_eng="act", pipe=True,
                kcomp=True)


def kernel(x, mask, w_qkv, w_out, bias_table):
    in_maps = host_prep(x, mask, w_qkv, w_out, bias_table)
    nc = _get_nc(WPC)
    res = run_bass_kernel_spmd(nc, in_maps, core_ids=list(range(NCORES)))
    core_outs = [res.results[i]["out"] for i in range(NCORES)]
    return assemble_output(core_outs)

